# revision 13
# baseline (speedup 1.0000x reference)
"""Trainium2 Bass kernel for nn_ChunkedMultiHeadCardPassingLayer.

Sharding: 8 cores = (batch b = core//2) x (T-half = core%2). Each core
processes 2048 contiguous tokens of one batch end-to-end; the only
cross-core dependency is the chunk-carry prefix, resolved with a 4KB
paired AllReduce.

v2: bf16 matmul operands + bf16 elementwise (2x DVE modes), W1@W2
fold so the MLP tail is 2 ACT + 1 DVE op, segmented reduces offloaded
to GPSIMD, sqrt via exp(-0.5*ln(x)) to stay on one ACT table set,
xt kept resident across phases, pjw prefetched before the collective.

Self-contained: hardcodes shapes; host-side prep is limited to slicing,
transposes, casts and tiny constant matrices.
"""
import os
os.environ.setdefault("JAX_PLATFORMS", "cpu")

import numpy as np
import ml_dtypes
from contextlib import ExitStack

import concourse.bacc as bacc
import concourse.mybir as mybir
import concourse.tile as tile
from concourse.bass_utils import run_bass_kernel_spmd

F32 = mybir.dt.float32
BF16 = mybir.dt.bfloat16
AX = mybir.AxisListType
ALU = mybir.AluOpType
ACTF = mybir.ActivationFunctionType

# problem constants
B, T, C = 4, 4096, 1024
H, CS = 16, 128
D = C // H            # 64
NCORES = 8
R = T // 2            # 2048 rows per core
NCH = R // CS         # 16 chunks per core
NG = C // 128         # 8 groups of (2 heads x 64)
NPG = NCH // 4        # 4 position groups of 512
EPS = 1e-5
P = 128


def _build(ncores, alpha, has_mark_b, has_gate_b, has_proj_b,
           has_carry_gb, has_b1, has_b2, has_ln_g, has_ln_b):
    assert not has_b1 and not has_b2, "MLP biases folded out; must be zero"
    nc = bacc.Bacc("TRN2", target_bir_lowering=False, debug=False,
                   num_devices=ncores)

    # ---------------- DRAM I/O ----------------
    xt_d = nc.dram_tensor("xt", [C, R], BF16, kind="ExternalInput")
    xn_d = nc.dram_tensor("xn", [R, C], F32, kind="ExternalInput")
    mkw_d = nc.dram_tensor("mkw", [C, C], BF16, kind="ExternalInput")
    gtw_d = nc.dram_tensor("gtw", [C, C], BF16, kind="ExternalInput")
    pjw_d = nc.dram_tensor("pjw", [C, C], BF16, kind="ExternalInput")
    mkb_d = nc.dram_tensor("mkb", [1, C], BF16, kind="ExternalInput")
    gtb_d = nc.dram_tensor("gtb", [1, C], BF16, kind="ExternalInput")
    pjb_d = nc.dram_tensor("pjb", [1, C], BF16, kind="ExternalInput")
    w1x_d = nc.dram_tensor("w1x", [2 * D, 2 * D], BF16, kind="ExternalInput")
    w1c_d = nc.dram_tensor("w1c", [2 * D, 2 * D], BF16, kind="ExternalInput")
    w12x_d = nc.dram_tensor("w12x", [2 * D, D], BF16, kind="ExternalInput")
    w12c_d = nc.dram_tensor("w12c", [2 * D, D], BF16, kind="ExternalInput")
    b1_d = nc.dram_tensor("b1c", [2 * D, 1], F32, kind="ExternalInput")
    w2_d = nc.dram_tensor("w2", [2 * D, D], BF16, kind="ExternalInput")
    b2_d = nc.dram_tensor("b2c", [D, 1], F32, kind="ExternalInput")
    ut_d = nc.dram_tensor("ut", [P, P], BF16, kind="ExternalInput")
    st_d = nc.dram_tensor("st", [P, P], BF16, kind="ExternalInput")
    l0_d = nc.dram_tensor("l0", [NCH, NCH], F32, kind="ExternalInput")
    eye_d = nc.dram_tensor("eyer", [P, P], BF16, kind="ExternalInput")
    csel_d = nc.dram_tensor("csel", [P, NCH * NCH], BF16,
                            kind="ExternalInput")
    onesr_d = nc.dram_tensor("onesr", [1, P], BF16, kind="ExternalInput")
    segm_d = nc.dram_tensor("segm", [1, 1], F32, kind="ExternalInput")
    usem_d = nc.dram_tensor("usem", [1, 1], F32, kind="ExternalInput")
    cgr_d = nc.dram_tensor("cgr", [NCH, D], F32, kind="ExternalInput")
    cbr_d = nc.dram_tensor("cbr", [NCH, D], F32, kind="ExternalInput")
    lgr_d = nc.dram_tensor("lgr", [P, C], F32, kind="ExternalInput")
    lbr_d = nc.dram_tensor("lbr", [P, C], F32, kind="ExternalInput")

    y_d = nc.dram_tensor("y", [R, C], F32, kind="ExternalOutput")

    lc_d = nc.dram_tensor("lc_spill", [R, C], BF16)   # local_cum spill
    cc_in = nc.dram_tensor("cc_in", [1, C], F32)
    cc_out = nc.dram_tensor("cc_out", [1, C], F32)

    groups = ([[i, i + 1] for i in range(0, ncores, 2)]
              if ncores > 1 else [[0]])

    with tile.TileContext(nc) as tc, ExitStack() as top:
        const_p = top.enter_context(tc.tile_pool(name="const", bufs=1))
        carr_p = top.enter_context(tc.tile_pool(name="carr", bufs=1))
        xt_p = top.enter_context(tc.tile_pool(name="xtp", bufs=1))
        pj_p = top.enter_context(tc.tile_pool(name="pjp", bufs=1))

        # ---------- constants ----------
        ut = const_p.tile([P, P], BF16)
        st = const_p.tile([P, P], BF16)
        l0 = const_p.tile([NCH, NCH], F32)
        eyer = const_p.tile([P, P], BF16)
        csel = const_p.tile([P, NCH * NCH], BF16)
        w1x = const_p.tile([2 * D, 2 * D], BF16)
        w1c = const_p.tile([2 * D, 2 * D], BF16)
        w12x = const_p.tile([2 * D, D], BF16)
        w12c = const_p.tile([2 * D, D], BF16)
        b1c = const_p.tile([2 * D, 1], F32)
        w2 = const_p.tile([2 * D, D], BF16)
        b2c = const_p.tile([D, 1], F32)
        segm = const_p.tile([1, 1], F32)
        usem = const_p.tile([1, 1], F32)
        for t_, d_ in ((ut, ut_d), (st, st_d), (l0, l0_d), (eyer, eye_d),
                       (csel, csel_d), (w1x, w1x_d), (w1c, w1c_d),
                       (w12x, w12x_d), (w12c, w12c_d),
                       (b1c, b1_d), (w2, w2_d), (b2c, b2_d), (segm, segm_d),
                       (usem, usem_d)):
            nc.sync.dma_start(t_[:], d_.ap())
        ones1r = const_p.tile([1, P], BF16)
        nc.sync.dma_start(ones1r[:], onesr_d.ap())
        ones1_16 = const_p.tile([1, NCH], F32)
        nc.vector.memset(ones1_16[:], 1.0)
        ones16_1 = const_p.tile([NCH, 1], F32)
        nc.vector.memset(ones16_1[:], 1.0)
        eps128 = const_p.tile([P, 1], F32)
        nc.vector.memset(eps128[:], EPS)
        if has_mark_b or has_gate_b or has_proj_b:
            mkb = const_p.tile([1, C], BF16)
            gtb = const_p.tile([1, C], BF16)
            pjb = const_p.tile([1, C], BF16)
            nc.sync.dma_start(mkb[:], mkb_d.ap())
            nc.sync.dma_start(gtb[:], gtb_d.ap())
            nc.sync.dma_start(pjb[:], pjb_d.ap())
        if has_carry_gb:
            cgr = const_p.tile([NCH, D], F32)
            cbr = const_p.tile([NCH, D], F32)
            nc.sync.dma_start(cgr[:], cgr_d.ap())
            nc.sync.dma_start(cbr[:], cbr_d.ap())

        cs_sb = carr_p.tile([NCH, C], F32)
        ncarry = carr_p.tile([NCH, C], BF16)

        # xt resident across all phases (phase 1 lhsT + phase 3 rhs)
        xt = []
        for g in range(NG):
            t_ = xt_p.tile([P, R], BF16, tag=f"xt{g}", name=f"xt{g}")
            nc.sync.dma_start(t_[:], xt_d.ap()[g * P:(g + 1) * P, :])
            xt.append(t_)

        # ================ phase 1: pm/gate/scan ================
        with tc.tile_pool(name="wgt", bufs=1) as wgt_p, \
             tc.tile_pool(name="ph1", bufs=3) as ph1_p, \
             tc.tile_pool(name="ps1", bufs=2, space="PSUM") as ps1_p, \
             tc.tile_pool(name="pslc", bufs=2, space="PSUM") as pslc_p, \
             tc.tile_pool(name="pscs", bufs=1, space="PSUM") as pscs_p:
            mkw, gtw = [], []
            for k in range(NG):
                mt = wgt_p.tile([P, C], BF16, tag=f"mk{k}", name=f"mk{k}")
                gt_ = wgt_p.tile([P, C], BF16, tag=f"gk{k}", name=f"gk{k}")
                nc.sync.dma_start(mt[:], mkw_d.ap()[k * P:(k + 1) * P, :])
                nc.sync.dma_start(gt_[:], gtw_d.ap()[k * P:(k + 1) * P, :])
                mkw.append(mt)
                gtw.append(gt_)

            cs_ps = pscs_p.tile([NCH, C], F32, tag="csps")
            for j in range(NCH):
                for n in range(2):
                    sl = slice(n * 512, (n + 1) * 512)
                    pm_ps = ps1_p.tile([P, 512], F32, tag="pm", name="pm_ps")
                    gt_ps = ps1_p.tile([P, 512], F32, tag="gt", name="gt_ps")
                    for k in range(NG):
                        lhs = xt[k][:, j * P:(j + 1) * P]
                        st_ = (k == 0)
                        sp = (k == NG - 1) and not (has_mark_b or has_gate_b)
                        nc.tensor.matmul(pm_ps[:], lhs, mkw[k][:, sl],
                                         start=st_, stop=sp)
                        nc.tensor.matmul(gt_ps[:], lhs, gtw[k][:, sl],
                                         start=st_, stop=sp)
                    if has_mark_b or has_gate_b:
                        nc.tensor.matmul(pm_ps[:], ones1r[:], mkb[:, sl],
                                         start=False, stop=True)
                        nc.tensor.matmul(gt_ps[:], ones1r[:], gtb[:, sl],
                                         start=False, stop=True)
                    gates = ph1_p.tile([P, 512], F32, tag="gates",
                                       name="gates")
                    nc.scalar.activation(gates[:], gt_ps[:], ACTF.Sigmoid)
                    gated = ph1_p.tile([P, 512], BF16, tag="gated",
                                       name="gated")
                    nc.vector.tensor_tensor(gated[:], gates[:], pm_ps[:],
                                            op=ALU.mult)
                    nc.tensor.matmul(cs_ps[:, sl],
                                     csel[:, j * NCH:(j + 1) * NCH],
                                     gated[:], start=(j == 0),
                                     stop=(j == NCH - 1))
                    lc_ps = pslc_p.tile([P, 512], F32, tag="lcps",
                                        name="lc_ps")
                    nc.tensor.matmul(lc_ps[:], ut[:], gated[:],
                                     start=True, stop=True)
                    lcs = ph1_p.tile([P, 512], BF16, tag="lcs", name="lcs")
                    nc.scalar.copy(lcs[:], lc_ps[:])
                    nc.sync.dma_start(
                        lc_d.ap()[j * P:(j + 1) * P, sl], lcs[:])
            nc.vector.tensor_copy(cs_sb[:], cs_ps[:])

        # prefetch proj weights + ln gains while collective runs
        pjw = []
        for k in range(NG):
            pt = pj_p.tile([P, C], BF16, tag=f"pj{k}", name=f"pj{k}")
            nc.sync.dma_start(pt[:], pjw_d.ap()[k * P:(k + 1) * P, :])
            pjw.append(pt)
        if has_ln_g:
            lgr = pj_p.tile([P, C], F32)
            nc.sync.dma_start(lgr[:], lgr_d.ap())
        if has_ln_b:
            lbr = pj_p.tile([P, C], F32)
            nc.sync.dma_start(lbr[:], lbr_d.ap())

        # ================ carries + collective ================
        with tc.tile_pool(name="car", bufs=1) as car_p, \
             tc.tile_pool(name="pscar", bufs=1, space="PSUM") as pscar_p:
            tot_ps = pscar_p.tile([1, C], F32, tag="tot")
            for n in range(2):
                sl = slice(n * 512, (n + 1) * 512)
                nc.tensor.matmul(tot_ps[:, sl], ones16_1[:], cs_sb[:, sl],
                                 start=True, stop=True)
            ccin_sb = car_p.tile([1, C], F32)
            nc.vector.tensor_scalar(ccin_sb[:], tot_ps[:], segm[:], None,
                                    op0=ALU.mult)
            nc.sync.dma_start(cc_in.ap(), ccin_sb[:])
            nc.gpsimd.collective_compute(
                "AllReduce", ALU.add, replica_groups=groups,
                ins=[cc_in.ap()], outs=[cc_out.ap()])
            base_sb = car_p.tile([1, C], F32)
            nc.sync.dma_start(base_sb[:], cc_out.ap())
            basem = car_p.tile([1, C], F32)
            nc.vector.tensor_scalar(basem[:], base_sb[:], usem[:], None,
                                    op0=ALU.mult)

            carx_ps = pscar_p.tile([NCH, C], F32, tag="carx")
            for n in range(2):
                sl = slice(n * 512, (n + 1) * 512)
                nc.tensor.matmul(carx_ps[:, sl], l0[:], cs_sb[:, sl],
                                 start=True, stop=False)
                nc.tensor.matmul(carx_ps[:, sl], ones1_16[:], basem[:, sl],
                                 start=False, stop=True)

            # ncarry = LN(carries) over d segments
            carr = car_p.tile([NCH, C], F32)
            nc.vector.tensor_copy(carr[:], carx_ps[:])
            c3 = carr[:].rearrange("p (h d) -> p h d", d=D)
            r1 = car_p.tile([NCH, H], F32)
            nc.vector.tensor_reduce(r1[:], c3, axis=AX.X, op=ALU.add)
            sqc = car_p.tile([NCH, C], F32)
            nc.vector.tensor_tensor(sqc[:], carr[:], carr[:], op=ALU.mult)
            r2 = car_p.tile([NCH, H], F32)
            nc.vector.tensor_reduce(r2[:], sqc[:].rearrange(
                "p (h d) -> p h d", d=D), axis=AX.X, op=ALU.add)
            mu = car_p.tile([NCH, H], F32)
            nc.vector.tensor_scalar(mu[:], r1[:], 1.0 / D, None, op0=ALU.mult)
            em2 = car_p.tile([NCH, H], F32)
            nc.vector.tensor_scalar(em2[:], r2[:], 1.0 / D, None,
                                    op0=ALU.mult)
            musq = car_p.tile([NCH, H], F32)
            nc.vector.tensor_tensor(musq[:], mu[:], mu[:], op=ALU.mult)
            var = car_p.tile([NCH, H], F32)
            nc.vector.tensor_tensor(var[:], em2[:], musq[:], op=ALU.subtract)
            eps16 = car_p.tile([NCH, 1], F32)
            nc.vector.memset(eps16[:], EPS)
            lnv = car_p.tile([NCH, H], F32)
            nc.scalar.activation(lnv[:], var[:], ACTF.Ln, bias=eps16[:])
            rstd = car_p.tile([NCH, H], F32)
            nc.scalar.activation(rstd[:], lnv[:], ACTF.Exp, scale=-0.5)
            mu_b = mu[:].unsqueeze(2).to_broadcast([NCH, H, D])
            rstd_b = rstd[:].unsqueeze(2).to_broadcast([NCH, H, D])
            cen = car_p.tile([NCH, C], F32)
            nc.vector.tensor_tensor(cen[:].rearrange("p (h d) -> p h d", d=D),
                                    c3, mu_b, op=ALU.subtract)
            if has_carry_gb:
                nrm = car_p.tile([NCH, C], F32)
                nc.vector.tensor_tensor(
                    nrm[:].rearrange("p (h d) -> p h d", d=D),
                    cen[:].rearrange("p (h d) -> p h d", d=D), rstd_b,
                    op=ALU.mult)
                cg_b = cgr[:].unsqueeze(1).to_broadcast([NCH, H, D])
                cb_b = cbr[:].unsqueeze(1).to_broadcast([NCH, H, D])
                nrm2 = car_p.tile([NCH, C], F32)
                nc.vector.tensor_tensor(
                    nrm2[:].rearrange("p (h d) -> p h d", d=D),
                    nrm[:].rearrange("p (h d) -> p h d", d=D), cg_b,
                    op=ALU.mult)
                nc.vector.tensor_tensor(
                    ncarry[:].rearrange("p (h d) -> p h d", d=D),
                    nrm2[:].rearrange("p (h d) -> p h d", d=D), cb_b,
                    op=ALU.add)
            else:
                nc.vector.tensor_tensor(
                    ncarry[:].rearrange("p (h d) -> p h d", d=D),
                    cen[:].rearrange("p (h d) -> p h d", d=D), rstd_b,
                    op=ALU.mult)

        # ===== phases 2-4, interleaved per position-group of 4 chunks =====
        HH = H // 2  # heads per column half
        with ExitStack() as late:
            big_p = late.enter_context(tc.tile_pool(name="bigpool", bufs=28))
            lcin_p = late.enter_context(tc.tile_pool(name="lcin", bufs=3))
            ph2_p = late.enter_context(tc.tile_pool(name="ph2", bufs=2))
            ph3_p = late.enter_context(tc.tile_pool(name="ph3", bufs=2))
            ph4_p = late.enter_context(tc.tile_pool(name="ph4", bufs=2))
            ps2_p = late.enter_context(
                tc.tile_pool(name="ps2", bufs=2, space="PSUM"))
            pstr_p = late.enter_context(
                tc.tile_pool(name="pstr", bufs=1, space="PSUM"))
            ps3_p = late.enter_context(
                tc.tile_pool(name="ps3", bufs=2, space="PSUM"))
            ps3b_p = late.enter_context(
                tc.tile_pool(name="ps3b", bufs=1, space="PSUM"))
            ps4_p = late.enter_context(
                tc.tile_pool(name="ps4", bufs=1, space="PSUM"))

            for pg in range(NPG):
                psl = slice(pg * 512, (pg + 1) * 512)
                cardsT = [None] * NG
                outT = [None] * NG
                for g in range(NG):
                    cardsT[g] = big_p.tile([P, 512], BF16, tag="bigtile",
                                           name=f"cardsT{pg}_{g}")

                # ---- phase 2: cards for the 4 chunks of this pg ----
                for jj in range(4):
                    j = pg * 4 + jj
                    ncrow = lcin_p.tile([1, C], BF16, tag="ncrow",
                                        name="ncrow", bufs=2)
                    nc.sync.dma_start(ncrow[:], ncarry[j:j + 1, :])
                    for n in range(2):
                        sl = slice(n * 512, (n + 1) * 512)
                        lcj = lcin_p.tile([P, 512], BF16, tag="lcin",
                                          name="lcj", bufs=4)
                        nc.sync.dma_start(lcj[:],
                                          lc_d.ap()[j * P:(j + 1) * P, sl])
                        cl_ps = ps2_p.tile([P, 512], F32, tag="clps",
                                           name="cl_ps")
                        nc.tensor.matmul(cl_ps[:], st[:], lcj[:],
                                         start=True, stop=False)
                        nc.tensor.matmul(cl_ps[:], ones1r[:],
                                         ncrow[0:1, sl],
                                         start=False, stop=True)
                        # segmented LN over d
                        sq = ph2_p.tile([P, 512], BF16, tag="sq", name="sq")
                        nc.scalar.square(sq[:], cl_ps[:])
                        r1c = ph2_p.tile([P, HH], F32, tag="r1c", name="r1c")
                        nc.vector.tensor_reduce(
                            r1c[:],
                            cl_ps[:].rearrange("p (h d) -> p h d", d=D),
                            axis=AX.X, op=ALU.add)
                        r2c = ph2_p.tile([P, HH], F32, tag="r2c", name="r2c")
                        nc.vector.tensor_reduce(
                            r2c[:], sq[:].rearrange("p (h d) -> p h d", d=D),
                            axis=AX.X, op=ALU.add)
                        muc = ph2_p.tile([P, HH], F32, tag="muc",
                                         name="muc")
                        nc.gpsimd.tensor_scalar_mul(muc[:], r1c[:], 1.0 / D)
                        musqc = ph2_p.tile([P, HH], F32, tag="musqc",
                                           name="musqc")
                        nc.gpsimd.tensor_tensor(musqc[:], muc[:], muc[:],
                                                op=ALU.mult)
                        em2c = ph2_p.tile([P, HH], F32, tag="em2c",
                                          name="em2c")
                        nc.gpsimd.tensor_scalar_mul(em2c[:], r2c[:], 1.0 / D)
                        varc = ph2_p.tile([P, HH], F32, tag="varc",
                                          name="varc")
                        nc.gpsimd.tensor_tensor(varc[:], em2c[:], musqc[:],
                                                op=ALU.subtract)
                        lnvc = ph2_p.tile([P, HH], F32, tag="lnvc",
                                          name="lnvc")
                        nc.scalar.activation(lnvc[:], varc[:], ACTF.Ln,
                                             bias=eps128[:])
                        rstdc = ph2_p.tile([P, HH], BF16, tag="rstdc",
                                           name="rstdc")
                        nc.scalar.activation(rstdc[:], lnvc[:], ACTF.Exp,
                                             scale=-0.5)
                        mu_bc = muc[:].unsqueeze(2).to_broadcast([P, HH, D])
                        rstd_bc = rstdc[:].unsqueeze(2).to_broadcast(
                            [P, HH, D])
                        cenc = ph2_p.tile([P, 512], BF16, tag="cenc",
                                          name="cenc")
                        nc.vector.tensor_tensor(
                            cenc[:].rearrange("p (h d) -> p h d", d=D),
                            cl_ps[:].rearrange("p (h d) -> p h d", d=D),
                            mu_bc, op=ALU.subtract)
                        cards = ph2_p.tile([P, 512], BF16, tag="cards",
                                           name="cards")
                        nc.gpsimd.tensor_tensor(
                            cards[:].rearrange("p (h d) -> p h d", d=D),
                            cenc[:].rearrange("p (h d) -> p h d", d=D),
                            rstd_bc, op=ALU.mult)
                        for gg in range(4):
                            g = n * 4 + gg
                            tr_ps = pstr_p.tile([P, P], BF16, tag="trps",
                                                name="tr_ps")
                            nc.tensor.transpose(
                                tr_ps[:], cards[:, gg * P:(gg + 1) * P],
                                eyer[:])
                            if gg % 2 == 0:
                                nc.scalar.copy(
                                    cardsT[g][:, jj * P:(jj + 1) * P],
                                    tr_ps[:])
                            else:
                                nc.vector.tensor_copy(
                                    cardsT[g][:, jj * P:(jj + 1) * P],
                                    tr_ps[:])

                # ---- phase 3: head MLP for this pg ----
                # ho = comb @ (W1@W2) + bump @ W2,
                # bump = alpha * u * exp(-u^2/2), u = comb @ W1 (+b1)
                for g in range(NG):
                    outT[g] = big_p.tile([P, 512], BF16, tag="bigtile",
                                         name=f"outT{pg}_{g}")
                for g in range(NG):
                    o2_ps = ps3b_p.tile([P, 512], F32, tag="o2",
                                        name="o2_ps")
                    for hh in range(2):
                        h = 2 * g + hh
                        off = hh * D
                        xg_r = xt[g][off:off + D, psl]
                        cd_r = cardsT[g][off:off + D, :]
                        h1_ps = ps3_p.tile([P, 512], F32, tag="h1",
                                           name="h1_ps")
                        nc.tensor.matmul(h1_ps[:], w1x[off:off + D, :], xg_r,
                                         start=True, stop=False)
                        nc.tensor.matmul(h1_ps[:], w1c[off:off + D, :], cd_r,
                                         start=False, stop=True)
                        sq3 = ph3_p.tile([P, 512], BF16, tag="sq3",
                                         name="sq3")
                        nc.scalar.square(sq3[:], h1_ps[:])
                        e3 = ph3_p.tile([P, 512], BF16, tag="e3", name="e3")
                        nc.scalar.activation(e3[:], sq3[:], ACTF.Exp,
                                             scale=-0.5)
                        wb = ph3_p.tile([P, 512], BF16, tag="wb", name="wb")
                        nc.vector.scalar_tensor_tensor(
                            wb[:], e3[:], float(alpha), h1_ps[:],
                            op0=ALU.mult, op1=ALU.mult)
                        o2v = o2_ps[off:off + D, :]
                        nc.tensor.matmul(o2v, w12x[off:off + D, :], xg_r,
                                         start=True, stop=False)
                        nc.tensor.matmul(o2v, w12c[off:off + D, :], cd_r,
                                         start=False, stop=False)
                        nc.tensor.matmul(o2v, w2[:], wb[:],
                                         start=False, stop=True)
                    nc.vector.tensor_copy(outT[g][:], o2_ps[:])

                # ---- phase 4: proj + LN + residual for this pg ----
                for tt in range(4):
                    t_i = pg * 4 + tt
                    col = tt * P
                    y_ps = ps4_p.tile([P, C], F32, tag="yps", name="y_ps")
                    for k in range(NG):
                        lhs = outT[k][:, col:col + P]
                        st_ = (k == 0)
                        sp = (k == NG - 1) and not has_proj_b
                        for n in range(2):
                            sl = slice(n * 512, (n + 1) * 512)
                            nc.tensor.matmul(y_ps[:, sl], lhs, pjw[k][:, sl],
                                             start=st_, stop=sp)
                    if has_proj_b:
                        for n in range(2):
                            sl = slice(n * 512, (n + 1) * 512)
                            nc.tensor.matmul(y_ps[:, sl], ones1r[:],
                                             pjb[:, sl],
                                             start=False, stop=True)
                    y_raw = ph4_p.tile([P, C], F32, tag="yraw", name="y_raw")
                    s1 = ph4_p.tile([P, 1], F32, tag="s1", name="s1")
                    nc.scalar.activation(y_raw[:], y_ps[:], ACTF.Copy,
                                         accum_out=s1[:])
                    sc4 = ph4_p.tile([P, C], BF16, tag="sc4", name="sc4",
                                     bufs=1)
                    s2 = ph4_p.tile([P, 1], F32, tag="s2", name="s2")
                    nc.scalar.activation(sc4[:], y_ps[:], ACTF.Square,
                                         scale=1.0 / 32.0, accum_out=s2[:])
                    m1 = ph4_p.tile([P, 1], F32, tag="m1", name="m1")
                    nc.vector.tensor_scalar(m1[:], s1[:], 1.0 / C, None,
                                            op0=ALU.mult)
                    msq = ph4_p.tile([P, 1], F32, tag="msq", name="msq")
                    nc.vector.tensor_tensor(msq[:], m1[:], m1[:],
                                            op=ALU.mult)
                    var4 = ph4_p.tile([P, 1], F32, tag="var4", name="var4")
                    nc.vector.tensor_tensor(var4[:], s2[:], msq[:],
                                            op=ALU.subtract)
                    lnv4 = ph4_p.tile([P, 1], F32, tag="lnv4", name="lnv4")
                    nc.scalar.activation(lnv4[:], var4[:], ACTF.Ln,
                                         bias=eps128[:])
                    rstd4 = ph4_p.tile([P, 1], F32, tag="rstd4",
                                       name="rstd4")
                    nc.scalar.activation(rstd4[:], lnv4[:], ACTF.Exp,
                                         scale=-0.5)
                    tnorm = ph4_p.tile([P, C], F32, tag="tnorm",
                                       name="tnorm")
                    nc.vector.tensor_scalar(tnorm[:], y_raw[:], m1[:],
                                            rstd4[:], op0=ALU.subtract,
                                            op1=ALU.mult)
                    if has_ln_g:
                        nc.vector.tensor_tensor(tnorm[:], tnorm[:], lgr[:],
                                                op=ALU.mult)
                    if has_ln_b:
                        nc.vector.tensor_tensor(tnorm[:], tnorm[:], lbr[:],
                                                op=ALU.add)
                    xa = ph4_p.tile([P, C], F32, tag="xa", name="xa")
                    nc.sync.dma_start(xa[:],
                                      xn_d.ap()[t_i * P:(t_i + 1) * P, :])
                    yout = ph4_p.tile([P, C], F32, tag="yout", name="yout")
                    nc.gpsimd.tensor_tensor(yout[:], tnorm[:], xa[:],
                                            op=ALU.add)
                    nc.sync.dma_start(y_d.ap()[t_i * P:(t_i + 1) * P, :],
                                      yout[:])

    nc.compile()
    return nc


_CACHE = {}


def _get_program(alpha, flags):
    key = (alpha, flags)
    if key not in _CACHE:
        _CACHE[key] = _build(NCORES, alpha, *flags)
    return _CACHE[key]


def _bf(x):
    return np.ascontiguousarray(x.astype(ml_dtypes.bfloat16))


def make_consts(W1, b1, card_g, card_b, carry_g, carry_b, ln_g, ln_b, W2):
    # fold card_g into the cards half of W1; card_b into b1
    W1xh = W1[:D, :]                     # [D, 2D]
    W1ch = card_g[:, None] * W1[D:, :]   # [D, 2D]
    b1f = (b1 + card_b @ W1[D:, :]).astype(np.float32)
    # W12 = W1 @ W2 (+ b1 folded at runtime via b2 path); bump handled apart
    W12x = (W1xh.astype(np.float64) @ W2.astype(np.float64)).astype(
        np.float32)
    W12c = (W1ch.astype(np.float64) @ W2.astype(np.float64)).astype(
        np.float32)
    ut = np.triu(np.ones((P, P), np.float32))
    stm = np.zeros((P, P), np.float32)
    for i in range(1, P):
        stm[i - 1, i] = 1.0
    l0 = np.triu(np.ones((NCH, NCH), np.float32), k=1)
    csel = np.zeros((P, NCH, NCH), np.float32)
    for j in range(NCH):
        csel[:, j, j] = 1.0
    csel = csel.reshape(P, NCH * NCH)
    return {
        "w1x": _bf(np.concatenate([W1xh, W1xh], 0)),
        "w1c": _bf(np.concatenate([W1ch, W1ch], 0)),
        "b1c": b1f[:, None],
        "w12x": _bf(np.concatenate([W12x, W12x], 0)),
        "w12c": _bf(np.concatenate([W12c, W12c], 0)),
        "ut": _bf(ut), "st": _bf(stm), "l0": l0, "csel": _bf(csel),
        "eyer": _bf(np.eye(P, dtype=np.float32)),
        "onesr": _bf(np.ones((1, P), np.float32)),
        "cgr": np.tile(carry_g[None, :], (NCH, 1)).astype(np.float32),
        "cbr": np.tile(carry_b[None, :], (NCH, 1)).astype(np.float32),
        "lgr": np.tile(ln_g[None, :], (P, 1)).astype(np.float32),
        "lbr": np.tile(ln_b[None, :], (P, 1)).astype(np.float32),
    }


def build_all(inputs):
    """Returns (nc, in_maps) for the 8 cores."""
    x = np.ascontiguousarray(np.asarray(inputs["x"], np.float32))
    mark_W = np.asarray(inputs["mark_W"], np.float32)
    mark_b = np.asarray(inputs["mark_b"], np.float32)
    gate_W = np.asarray(inputs["gate_W"], np.float32)
    gate_b = np.asarray(inputs["gate_b"], np.float32)
    carry_g = np.asarray(inputs["carry_g"], np.float32)
    carry_b = np.asarray(inputs["carry_b"], np.float32)
    card_g = np.asarray(inputs["card_g"], np.float32)
    card_b = np.asarray(inputs["card_b"], np.float32)
    W1 = np.asarray(inputs["W1"], np.float32)
    b1 = np.asarray(inputs["b1"], np.float32)
    alpha = float(np.asarray(inputs["alpha"]))
    W2 = np.asarray(inputs["W2"], np.float32)
    b2 = np.asarray(inputs["b2"], np.float32)
    proj_W = np.asarray(inputs["proj_W"], np.float32)
    proj_b = np.asarray(inputs["proj_b"], np.float32)
    ln_g = np.asarray(inputs["ln_g"], np.float32)
    ln_b = np.asarray(inputs["ln_b"], np.float32)

    has_carry_gb = bool(np.any(carry_g != 1.0) or np.any(carry_b != 0.0))
    b1f = b1 + card_b @ W1[D:, :]
    flags = (bool(np.any(mark_b)), bool(np.any(gate_b)), bool(np.any(proj_b)),
             has_carry_gb, bool(np.any(b1f)), bool(np.any(b2)),
             bool(np.any(ln_g != 1.0)), bool(np.any(ln_b)))
    nc = _get_program(alpha, flags)

    common = make_consts(W1, b1, card_g, card_b, carry_g, carry_b,
                         ln_g, ln_b, W2)
    common.update({
        "mkw": _bf(mark_W), "gtw": _bf(gate_W), "pjw": _bf(proj_W),
        "mkb": _bf(mark_b[None, :]), "gtb": _bf(gate_b[None, :]),
        "pjb": _bf(proj_b[None, :]),
        "w2": _bf(W2), "b2c": b2[:, None].astype(np.float32),
    })
    in_maps = []
    for c in range(NCORES):
        b, half = c // 2, c % 2
        xs = x[b, half * R:(half + 1) * R, :]
        m = dict(common)
        m["xn"] = np.ascontiguousarray(xs)
        m["xt"] = _bf(xs.T)
        m["segm"] = np.array([[1.0 - half]], np.float32)
        m["usem"] = np.array([[float(half)]], np.float32)
        in_maps.append(m)
    return nc, in_maps


def kernel(**inputs):
    nc, in_maps = build_all(inputs)
    res = run_bass_kernel_spmd(nc, in_maps, list(range(NCORES)))
    out = np.empty((B, T, C), np.float32)
    for c in range(NCORES):
        b, half = c // 2, c % 2
        out[b, half * R:(half + 1) * R, :] = res.results[c]["y"]
    return out


# revision 15
# speedup vs baseline: 1.0426x; 1.0426x over previous
"""Trainium2 Bass kernel for nn_ChunkedMultiHeadCardPassingLayer.

Sharding: 8 cores = (batch b = core//2) x (T-half = core%2). Each core
processes 2048 contiguous tokens of one batch end-to-end; the only
cross-core dependency is the chunk-carry prefix, resolved with a 4KB
paired AllReduce.

v2: bf16 matmul operands + bf16 elementwise (2x DVE modes), W1@W2
fold so the MLP tail is 2 ACT + 1 DVE op, segmented reduces offloaded
to GPSIMD, sqrt via exp(-0.5*ln(x)) to stay on one ACT table set,
xt kept resident across phases, pjw prefetched before the collective.

Self-contained: hardcodes shapes; host-side prep is limited to slicing,
transposes, casts and tiny constant matrices.
"""
import os
os.environ.setdefault("JAX_PLATFORMS", "cpu")

import numpy as np
import ml_dtypes
from contextlib import ExitStack

import concourse.bacc as bacc
import concourse.mybir as mybir
import concourse.tile as tile
from concourse.bass_utils import run_bass_kernel_spmd

F32 = mybir.dt.float32
BF16 = mybir.dt.bfloat16
AX = mybir.AxisListType
ALU = mybir.AluOpType
ACTF = mybir.ActivationFunctionType

# problem constants
B, T, C = 4, 4096, 1024
H, CS = 16, 128
D = C // H            # 64
NCORES = 8
R = T // 2            # 2048 rows per core
NCH = R // CS         # 16 chunks per core
NG = C // 128         # 8 groups of (2 heads x 64)
NPG = NCH // 4        # 4 position groups of 512
EPS = 1e-5
P = 128


def _build(ncores, alpha, has_mark_b, has_gate_b, has_proj_b,
           has_carry_gb, has_b1, has_b2, has_ln_g, has_ln_b):
    assert not has_b1 and not has_b2, "MLP biases folded out; must be zero"
    nc = bacc.Bacc("TRN2", target_bir_lowering=False, debug=False,
                   num_devices=ncores)

    # ---------------- DRAM I/O ----------------
    xt_d = nc.dram_tensor("xt", [C, R], BF16, kind="ExternalInput")
    xn_d = nc.dram_tensor("xn", [R, C], F32, kind="ExternalInput")
    mkw_d = nc.dram_tensor("mkw", [C, C], BF16, kind="ExternalInput")
    gtw_d = nc.dram_tensor("gtw", [C, C], BF16, kind="ExternalInput")
    pjw_d = nc.dram_tensor("pjw", [C, C], BF16, kind="ExternalInput")
    mkb_d = nc.dram_tensor("mkb", [1, C], BF16, kind="ExternalInput")
    gtb_d = nc.dram_tensor("gtb", [1, C], BF16, kind="ExternalInput")
    pjb_d = nc.dram_tensor("pjb", [1, C], BF16, kind="ExternalInput")
    w1x_d = nc.dram_tensor("w1x", [2 * D, 2 * D], BF16, kind="ExternalInput")
    w1c_d = nc.dram_tensor("w1c", [2 * D, 2 * D], BF16, kind="ExternalInput")
    w12x_d = nc.dram_tensor("w12x", [2 * D, D], BF16, kind="ExternalInput")
    w12c_d = nc.dram_tensor("w12c", [2 * D, D], BF16, kind="ExternalInput")
    b1_d = nc.dram_tensor("b1c", [2 * D, 1], F32, kind="ExternalInput")
    w2_d = nc.dram_tensor("w2", [2 * D, D], BF16, kind="ExternalInput")
    b2_d = nc.dram_tensor("b2c", [D, 1], F32, kind="ExternalInput")
    ut_d = nc.dram_tensor("ut", [P, P], BF16, kind="ExternalInput")
    st_d = nc.dram_tensor("st", [P, P], BF16, kind="ExternalInput")
    l0_d = nc.dram_tensor("l0", [NCH, NCH], F32, kind="ExternalInput")
    eye_d = nc.dram_tensor("eyer", [P, P], BF16, kind="ExternalInput")
    csel_d = nc.dram_tensor("csel", [P, NCH * NCH], BF16,
                            kind="ExternalInput")
    onesr_d = nc.dram_tensor("onesr", [1, P], BF16, kind="ExternalInput")
    segm_d = nc.dram_tensor("segm", [1, 1], F32, kind="ExternalInput")
    usem_d = nc.dram_tensor("usem", [1, 1], F32, kind="ExternalInput")
    cgr_d = nc.dram_tensor("cgr", [NCH, D], F32, kind="ExternalInput")
    cbr_d = nc.dram_tensor("cbr", [NCH, D], F32, kind="ExternalInput")
    lgr_d = nc.dram_tensor("lgr", [P, C], F32, kind="ExternalInput")
    lbr_d = nc.dram_tensor("lbr", [P, C], F32, kind="ExternalInput")

    y_d = nc.dram_tensor("y", [R, C], F32, kind="ExternalOutput")

    lc_d = nc.dram_tensor("lc_spill", [R, C], BF16)   # local_cum spill
    cc_in = nc.dram_tensor("cc_in", [1, C], F32)
    cc_out = nc.dram_tensor("cc_out", [1, C], F32)

    groups = ([[i, i + 1] for i in range(0, ncores, 2)]
              if ncores > 1 else [[0]])

    with tile.TileContext(nc) as tc, ExitStack() as top:
        const_p = top.enter_context(tc.tile_pool(name="const", bufs=1))
        carr_p = top.enter_context(tc.tile_pool(name="carr", bufs=1))
        xt_p = top.enter_context(tc.tile_pool(name="xtp", bufs=1))
        pj_p = top.enter_context(tc.tile_pool(name="pjp", bufs=1))

        # ---------- constants ----------
        ut = const_p.tile([P, P], BF16)
        st = const_p.tile([P, P], BF16)
        l0 = const_p.tile([NCH, NCH], F32)
        eyer = const_p.tile([P, P], BF16)
        csel = const_p.tile([P, NCH * NCH], BF16)
        w1x = const_p.tile([2 * D, 2 * D], BF16)
        w1c = const_p.tile([2 * D, 2 * D], BF16)
        w12x = const_p.tile([2 * D, D], BF16)
        w12c = const_p.tile([2 * D, D], BF16)
        b1c = const_p.tile([2 * D, 1], F32)
        w2 = const_p.tile([2 * D, D], BF16)
        b2c = const_p.tile([D, 1], F32)
        segm = const_p.tile([1, 1], F32)
        usem = const_p.tile([1, 1], F32)
        for t_, d_ in ((ut, ut_d), (st, st_d), (l0, l0_d), (eyer, eye_d),
                       (csel, csel_d), (w1x, w1x_d), (w1c, w1c_d),
                       (w12x, w12x_d), (w12c, w12c_d),
                       (b1c, b1_d), (w2, w2_d), (b2c, b2_d), (segm, segm_d),
                       (usem, usem_d)):
            nc.sync.dma_start(t_[:], d_.ap())
        ones1r = const_p.tile([1, P], BF16)
        nc.sync.dma_start(ones1r[:], onesr_d.ap())
        ones1_16 = const_p.tile([1, NCH], F32)
        nc.vector.memset(ones1_16[:], 1.0)
        ones16_1 = const_p.tile([NCH, 1], F32)
        nc.vector.memset(ones16_1[:], 1.0)
        eps128 = const_p.tile([P, 1], F32)
        nc.vector.memset(eps128[:], EPS)
        if has_mark_b or has_gate_b or has_proj_b:
            mkb = const_p.tile([1, C], BF16)
            gtb = const_p.tile([1, C], BF16)
            pjb = const_p.tile([1, C], BF16)
            nc.sync.dma_start(mkb[:], mkb_d.ap())
            nc.sync.dma_start(gtb[:], gtb_d.ap())
            nc.sync.dma_start(pjb[:], pjb_d.ap())
        if has_carry_gb:
            cgr = const_p.tile([NCH, D], F32)
            cbr = const_p.tile([NCH, D], F32)
            nc.sync.dma_start(cgr[:], cgr_d.ap())
            nc.sync.dma_start(cbr[:], cbr_d.ap())

        cs_sb = carr_p.tile([NCH, C], F32)
        ncarry = carr_p.tile([NCH, C], BF16)

        # xt resident across all phases (phase 1 lhsT + phase 3 rhs)
        xt = []
        for g in range(NG):
            t_ = xt_p.tile([P, R], BF16, tag=f"xt{g}", name=f"xt{g}")
            nc.sync.dma_start(t_[:], xt_d.ap()[g * P:(g + 1) * P, :])
            xt.append(t_)

        # ================ phase 1: pm/gate/scan ================
        with tc.tile_pool(name="wgt", bufs=1) as wgt_p, \
             tc.tile_pool(name="ph1", bufs=3) as ph1_p, \
             tc.tile_pool(name="ps1", bufs=2, space="PSUM") as ps1_p, \
             tc.tile_pool(name="pslc", bufs=2, space="PSUM") as pslc_p, \
             tc.tile_pool(name="pscs", bufs=1, space="PSUM") as pscs_p:
            mkw, gtw = [], []
            for k in range(NG):
                mt = wgt_p.tile([P, C], BF16, tag=f"mk{k}", name=f"mk{k}")
                gt_ = wgt_p.tile([P, C], BF16, tag=f"gk{k}", name=f"gk{k}")
                nc.sync.dma_start(mt[:], mkw_d.ap()[k * P:(k + 1) * P, :])
                nc.sync.dma_start(gt_[:], gtw_d.ap()[k * P:(k + 1) * P, :])
                mkw.append(mt)
                gtw.append(gt_)

            cs_ps = pscs_p.tile([NCH, C], F32, tag="csps")
            for j in range(NCH):
                for n in range(2):
                    sl = slice(n * 512, (n + 1) * 512)
                    pm_ps = ps1_p.tile([P, 512], F32, tag="pm", name="pm_ps")
                    gt_ps = ps1_p.tile([P, 512], F32, tag="gt", name="gt_ps")
                    for k in range(NG):
                        lhs = xt[k][:, j * P:(j + 1) * P]
                        st_ = (k == 0)
                        sp = (k == NG - 1) and not (has_mark_b or has_gate_b)
                        nc.tensor.matmul(pm_ps[:], lhs, mkw[k][:, sl],
                                         start=st_, stop=sp)
                        nc.tensor.matmul(gt_ps[:], lhs, gtw[k][:, sl],
                                         start=st_, stop=sp)
                    if has_mark_b or has_gate_b:
                        nc.tensor.matmul(pm_ps[:], ones1r[:], mkb[:, sl],
                                         start=False, stop=True)
                        nc.tensor.matmul(gt_ps[:], ones1r[:], gtb[:, sl],
                                         start=False, stop=True)
                    gates = ph1_p.tile([P, 512], F32, tag="gates",
                                       name="gates")
                    nc.scalar.activation(gates[:], gt_ps[:], ACTF.Sigmoid)
                    gated = ph1_p.tile([P, 512], BF16, tag="gated",
                                       name="gated")
                    nc.vector.tensor_tensor(gated[:], gates[:], pm_ps[:],
                                            op=ALU.mult)
                    nc.tensor.matmul(cs_ps[:, sl],
                                     csel[:, j * NCH:(j + 1) * NCH],
                                     gated[:], start=(j == 0),
                                     stop=(j == NCH - 1))
                    lc_ps = pslc_p.tile([P, 512], F32, tag="lcps",
                                        name="lc_ps")
                    nc.tensor.matmul(lc_ps[:], ut[:], gated[:],
                                     start=True, stop=True)
                    lcs = ph1_p.tile([P, 512], BF16, tag="lcs", name="lcs")
                    nc.scalar.copy(lcs[:], lc_ps[:])
                    nc.sync.dma_start(
                        lc_d.ap()[j * P:(j + 1) * P, sl], lcs[:])
            nc.vector.tensor_copy(cs_sb[:], cs_ps[:])

        # prefetch proj weights + ln gains while collective runs
        pjw = []
        for k in range(NG):
            pt = pj_p.tile([P, C], BF16, tag=f"pj{k}", name=f"pj{k}")
            nc.sync.dma_start(pt[:], pjw_d.ap()[k * P:(k + 1) * P, :])
            pjw.append(pt)
        if has_ln_g:
            lgr = pj_p.tile([P, C], F32)
            nc.sync.dma_start(lgr[:], lgr_d.ap())
        if has_ln_b:
            lbr = pj_p.tile([P, C], F32)
            nc.sync.dma_start(lbr[:], lbr_d.ap())

        # ================ carries + collective ================
        with tc.tile_pool(name="car", bufs=1) as car_p, \
             tc.tile_pool(name="pscar", bufs=1, space="PSUM") as pscar_p:
            tot_ps = pscar_p.tile([1, C], F32, tag="tot")
            for n in range(2):
                sl = slice(n * 512, (n + 1) * 512)
                nc.tensor.matmul(tot_ps[:, sl], ones16_1[:], cs_sb[:, sl],
                                 start=True, stop=True)
            ccin_sb = car_p.tile([1, C], F32)
            nc.vector.tensor_scalar(ccin_sb[:], tot_ps[:], segm[:], None,
                                    op0=ALU.mult)
            nc.sync.dma_start(cc_in.ap(), ccin_sb[:])
            nc.gpsimd.collective_compute(
                "AllReduce", ALU.add, replica_groups=groups,
                ins=[cc_in.ap()], outs=[cc_out.ap()])
            base_sb = car_p.tile([1, C], F32)
            nc.sync.dma_start(base_sb[:], cc_out.ap())
            basem = car_p.tile([1, C], F32)
            nc.vector.tensor_scalar(basem[:], base_sb[:], usem[:], None,
                                    op0=ALU.mult)

            carx_ps = pscar_p.tile([NCH, C], F32, tag="carx")
            for n in range(2):
                sl = slice(n * 512, (n + 1) * 512)
                nc.tensor.matmul(carx_ps[:, sl], l0[:], cs_sb[:, sl],
                                 start=True, stop=False)
                nc.tensor.matmul(carx_ps[:, sl], ones1_16[:], basem[:, sl],
                                 start=False, stop=True)

            # ncarry = LN(carries) over d segments
            carr = car_p.tile([NCH, C], F32)
            nc.vector.tensor_copy(carr[:], carx_ps[:])
            c3 = carr[:].rearrange("p (h d) -> p h d", d=D)
            r1 = car_p.tile([NCH, H], F32)
            nc.vector.tensor_reduce(r1[:], c3, axis=AX.X, op=ALU.add)
            sqc = car_p.tile([NCH, C], F32)
            nc.vector.tensor_tensor(sqc[:], carr[:], carr[:], op=ALU.mult)
            r2 = car_p.tile([NCH, H], F32)
            nc.vector.tensor_reduce(r2[:], sqc[:].rearrange(
                "p (h d) -> p h d", d=D), axis=AX.X, op=ALU.add)
            mu = car_p.tile([NCH, H], F32)
            nc.vector.tensor_scalar(mu[:], r1[:], 1.0 / D, None, op0=ALU.mult)
            em2 = car_p.tile([NCH, H], F32)
            nc.vector.tensor_scalar(em2[:], r2[:], 1.0 / D, None,
                                    op0=ALU.mult)
            musq = car_p.tile([NCH, H], F32)
            nc.vector.tensor_tensor(musq[:], mu[:], mu[:], op=ALU.mult)
            var = car_p.tile([NCH, H], F32)
            nc.vector.tensor_tensor(var[:], em2[:], musq[:], op=ALU.subtract)
            eps16 = car_p.tile([NCH, 1], F32)
            nc.vector.memset(eps16[:], EPS)
            sd = car_p.tile([NCH, H], F32)
            nc.scalar.activation(sd[:], var[:], ACTF.Sqrt, bias=eps16[:])
            rstd = car_p.tile([NCH, H], F32)
            nc.vector.reciprocal(rstd[:], sd[:])
            mu_b = mu[:].unsqueeze(2).to_broadcast([NCH, H, D])
            rstd_b = rstd[:].unsqueeze(2).to_broadcast([NCH, H, D])
            cen = car_p.tile([NCH, C], F32)
            nc.vector.tensor_tensor(cen[:].rearrange("p (h d) -> p h d", d=D),
                                    c3, mu_b, op=ALU.subtract)
            if has_carry_gb:
                nrm = car_p.tile([NCH, C], F32)
                nc.vector.tensor_tensor(
                    nrm[:].rearrange("p (h d) -> p h d", d=D),
                    cen[:].rearrange("p (h d) -> p h d", d=D), rstd_b,
                    op=ALU.mult)
                cg_b = cgr[:].unsqueeze(1).to_broadcast([NCH, H, D])
                cb_b = cbr[:].unsqueeze(1).to_broadcast([NCH, H, D])
                nrm2 = car_p.tile([NCH, C], F32)
                nc.vector.tensor_tensor(
                    nrm2[:].rearrange("p (h d) -> p h d", d=D),
                    nrm[:].rearrange("p (h d) -> p h d", d=D), cg_b,
                    op=ALU.mult)
                nc.vector.tensor_tensor(
                    ncarry[:].rearrange("p (h d) -> p h d", d=D),
                    nrm2[:].rearrange("p (h d) -> p h d", d=D), cb_b,
                    op=ALU.add)
            else:
                nc.vector.tensor_tensor(
                    ncarry[:].rearrange("p (h d) -> p h d", d=D),
                    cen[:].rearrange("p (h d) -> p h d", d=D), rstd_b,
                    op=ALU.mult)

        # ===== phases 2-4, interleaved per position-group of 4 chunks =====
        HH = H // 2  # heads per column half
        with ExitStack() as late:
            big_p = late.enter_context(tc.tile_pool(name="bigpool", bufs=28))
            lcin_p = late.enter_context(tc.tile_pool(name="lcin", bufs=3))
            ph2_p = late.enter_context(tc.tile_pool(name="ph2", bufs=2))
            ph3_p = late.enter_context(tc.tile_pool(name="ph3", bufs=2))
            ph4_p = late.enter_context(tc.tile_pool(name="ph4", bufs=2))
            ps2_p = late.enter_context(
                tc.tile_pool(name="ps2", bufs=2, space="PSUM"))
            pstr_p = late.enter_context(
                tc.tile_pool(name="pstr", bufs=1, space="PSUM"))
            ps3_p = late.enter_context(
                tc.tile_pool(name="ps3", bufs=2, space="PSUM"))
            ps3b_p = late.enter_context(
                tc.tile_pool(name="ps3b", bufs=1, space="PSUM"))
            ps4_p = late.enter_context(
                tc.tile_pool(name="ps4", bufs=1, space="PSUM"))

            for pg in range(NPG):
                psl = slice(pg * 512, (pg + 1) * 512)
                cardsT = [None] * NG
                outT = [None] * NG
                for g in range(NG):
                    cardsT[g] = big_p.tile([P, 512], BF16, tag="bigtile",
                                           name=f"cardsT{pg}_{g}")

                # ---- phase 2: cards for the 4 chunks of this pg ----
                for jj in range(4):
                    j = pg * 4 + jj
                    ncrow = lcin_p.tile([1, C], BF16, tag="ncrow",
                                        name="ncrow", bufs=2)
                    nc.sync.dma_start(ncrow[:], ncarry[j:j + 1, :])
                    for n in range(2):
                        sl = slice(n * 512, (n + 1) * 512)
                        lcj = lcin_p.tile([P, 512], BF16, tag="lcin",
                                          name="lcj", bufs=4)
                        nc.sync.dma_start(lcj[:],
                                          lc_d.ap()[j * P:(j + 1) * P, sl])
                        cl_ps = ps2_p.tile([P, 512], F32, tag="clps",
                                           name="cl_ps")
                        nc.tensor.matmul(cl_ps[:], st[:], lcj[:],
                                         start=True, stop=False)
                        nc.tensor.matmul(cl_ps[:], ones1r[:],
                                         ncrow[0:1, sl],
                                         start=False, stop=True)
                        # segmented LN over d
                        sq = ph2_p.tile([P, 512], BF16, tag="sq", name="sq")
                        nc.scalar.square(sq[:], cl_ps[:])
                        r1c = ph2_p.tile([P, HH], F32, tag="r1c", name="r1c")
                        nc.vector.tensor_reduce(
                            r1c[:],
                            cl_ps[:].rearrange("p (h d) -> p h d", d=D),
                            axis=AX.X, op=ALU.add)
                        r2c = ph2_p.tile([P, HH], F32, tag="r2c", name="r2c")
                        nc.vector.tensor_reduce(
                            r2c[:], sq[:].rearrange("p (h d) -> p h d", d=D),
                            axis=AX.X, op=ALU.add)
                        muc = ph2_p.tile([P, HH], F32, tag="muc",
                                         name="muc")
                        nc.vector.tensor_scalar(muc[:], r1c[:], 1.0 / D,
                                                None, op0=ALU.mult)
                        musqc = ph2_p.tile([P, HH], F32, tag="musqc",
                                           name="musqc")
                        nc.vector.tensor_tensor(musqc[:], muc[:], muc[:],
                                                op=ALU.mult)
                        varc = ph2_p.tile([P, HH], F32, tag="varc",
                                          name="varc")
                        nc.vector.scalar_tensor_tensor(
                            varc[:], r2c[:], 1.0 / D, musqc[:],
                            op0=ALU.mult, op1=ALU.subtract)
                        sdc = ph2_p.tile([P, HH], F32, tag="sdc",
                                         name="sdc")
                        nc.scalar.activation(sdc[:], varc[:], ACTF.Sqrt,
                                             bias=eps128[:])
                        rstdc = ph2_p.tile([P, HH], F32, tag="rstdc",
                                           name="rstdc")
                        nc.vector.reciprocal(rstdc[:], sdc[:])
                        mu_bc = muc[:].unsqueeze(2).to_broadcast([P, HH, D])
                        rstd_bc = rstdc[:].unsqueeze(2).to_broadcast(
                            [P, HH, D])
                        cenc = ph2_p.tile([P, 512], BF16, tag="cenc",
                                          name="cenc")
                        nc.vector.tensor_tensor(
                            cenc[:].rearrange("p (h d) -> p h d", d=D),
                            cl_ps[:].rearrange("p (h d) -> p h d", d=D),
                            mu_bc, op=ALU.subtract)
                        cards = ph2_p.tile([P, 512], BF16, tag="cards",
                                           name="cards")
                        nc.gpsimd.tensor_tensor(
                            cards[:].rearrange("p (h d) -> p h d", d=D),
                            cenc[:].rearrange("p (h d) -> p h d", d=D),
                            rstd_bc, op=ALU.mult)
                        for gg in range(4):
                            g = n * 4 + gg
                            tr_ps = pstr_p.tile([P, P], BF16, tag="trps",
                                                name="tr_ps")
                            nc.tensor.transpose(
                                tr_ps[:], cards[:, gg * P:(gg + 1) * P],
                                eyer[:])
                            if gg % 2 == 0:
                                nc.scalar.copy(
                                    cardsT[g][:, jj * P:(jj + 1) * P],
                                    tr_ps[:])
                            else:
                                nc.vector.tensor_copy(
                                    cardsT[g][:, jj * P:(jj + 1) * P],
                                    tr_ps[:])

                # ---- phase 3: head MLP for this pg ----
                # ho = comb @ (W1@W2) + bump @ W2,
                # bump = alpha * u * exp(-u^2/2), u = comb @ W1 (+b1)
                for g in range(NG):
                    outT[g] = big_p.tile([P, 512], BF16, tag="bigtile",
                                         name=f"outT{pg}_{g}")
                for g in range(NG):
                    o2_ps = ps3b_p.tile([P, 512], F32, tag="o2",
                                        name="o2_ps")
                    for hh in range(2):
                        h = 2 * g + hh
                        off = hh * D
                        xg_r = xt[g][off:off + D, psl]
                        cd_r = cardsT[g][off:off + D, :]
                        h1_ps = ps3_p.tile([P, 512], F32, tag="h1",
                                           name="h1_ps")
                        nc.tensor.matmul(h1_ps[:], w1x[off:off + D, :], xg_r,
                                         start=True, stop=False)
                        nc.tensor.matmul(h1_ps[:], w1c[off:off + D, :], cd_r,
                                         start=False, stop=True)
                        sq3 = ph3_p.tile([P, 512], BF16, tag="sq3",
                                         name="sq3")
                        nc.scalar.square(sq3[:], h1_ps[:])
                        e3 = ph3_p.tile([P, 512], BF16, tag="e3", name="e3")
                        nc.scalar.activation(e3[:], sq3[:], ACTF.Exp,
                                             scale=-0.5)
                        wb = ph3_p.tile([P, 512], BF16, tag="wb", name="wb")
                        nc.vector.scalar_tensor_tensor(
                            wb[:], e3[:], float(alpha), h1_ps[:],
                            op0=ALU.mult, op1=ALU.mult)
                        o2v = o2_ps[off:off + D, :]
                        nc.tensor.matmul(o2v, w12x[off:off + D, :], xg_r,
                                         start=True, stop=False)
                        nc.tensor.matmul(o2v, w12c[off:off + D, :], cd_r,
                                         start=False, stop=False)
                        nc.tensor.matmul(o2v, w2[:], wb[:],
                                         start=False, stop=True)
                    nc.vector.tensor_copy(outT[g][:], o2_ps[:])

                # ---- phase 4: proj + LN + residual for this pg ----
                for tt in range(4):
                    t_i = pg * 4 + tt
                    col = tt * P
                    y_ps = ps4_p.tile([P, C], F32, tag="yps", name="y_ps")
                    for k in range(NG):
                        lhs = outT[k][:, col:col + P]
                        st_ = (k == 0)
                        sp = (k == NG - 1) and not has_proj_b
                        for n in range(2):
                            sl = slice(n * 512, (n + 1) * 512)
                            nc.tensor.matmul(y_ps[:, sl], lhs, pjw[k][:, sl],
                                             start=st_, stop=sp)
                    if has_proj_b:
                        for n in range(2):
                            sl = slice(n * 512, (n + 1) * 512)
                            nc.tensor.matmul(y_ps[:, sl], ones1r[:],
                                             pjb[:, sl],
                                             start=False, stop=True)
                    y_raw = ph4_p.tile([P, C], F32, tag="yraw", name="y_raw")
                    s1 = ph4_p.tile([P, 1], F32, tag="s1", name="s1")
                    nc.scalar.activation(y_raw[:], y_ps[:], ACTF.Copy,
                                         accum_out=s1[:])
                    sc4 = ph4_p.tile([P, C], BF16, tag="sc4", name="sc4",
                                     bufs=1)
                    s2 = ph4_p.tile([P, 1], F32, tag="s2", name="s2")
                    nc.scalar.activation(sc4[:], y_ps[:], ACTF.Square,
                                         scale=1.0 / 32.0, accum_out=s2[:])
                    m1 = ph4_p.tile([P, 1], F32, tag="m1", name="m1")
                    nc.vector.tensor_scalar(m1[:], s1[:], 1.0 / C, None,
                                            op0=ALU.mult)
                    msq = ph4_p.tile([P, 1], F32, tag="msq", name="msq")
                    nc.vector.tensor_tensor(msq[:], m1[:], m1[:],
                                            op=ALU.mult)
                    var4 = ph4_p.tile([P, 1], F32, tag="var4", name="var4")
                    nc.vector.tensor_tensor(var4[:], s2[:], msq[:],
                                            op=ALU.subtract)
                    sd4 = ph4_p.tile([P, 1], F32, tag="sd4", name="sd4")
                    nc.scalar.activation(sd4[:], var4[:], ACTF.Sqrt,
                                         bias=eps128[:])
                    rstd4 = ph4_p.tile([P, 1], F32, tag="rstd4",
                                       name="rstd4")
                    nc.vector.reciprocal(rstd4[:], sd4[:])
                    tnorm = ph4_p.tile([P, C], F32, tag="tnorm",
                                       name="tnorm")
                    nc.vector.tensor_scalar(tnorm[:], y_raw[:], m1[:],
                                            rstd4[:], op0=ALU.subtract,
                                            op1=ALU.mult)
                    if has_ln_g:
                        nc.vector.tensor_tensor(tnorm[:], tnorm[:], lgr[:],
                                                op=ALU.mult)
                    if has_ln_b:
                        nc.vector.tensor_tensor(tnorm[:], tnorm[:], lbr[:],
                                                op=ALU.add)
                    xa = ph4_p.tile([P, C], F32, tag="xa", name="xa")
                    nc.sync.dma_start(xa[:],
                                      xn_d.ap()[t_i * P:(t_i + 1) * P, :])
                    yout = ph4_p.tile([P, C], F32, tag="yout", name="yout")
                    nc.gpsimd.tensor_tensor(yout[:], tnorm[:], xa[:],
                                            op=ALU.add)
                    nc.sync.dma_start(y_d.ap()[t_i * P:(t_i + 1) * P, :],
                                      yout[:])

    nc.compile()
    return nc


_CACHE = {}


def _get_program(alpha, flags):
    key = (alpha, flags)
    if key not in _CACHE:
        _CACHE[key] = _build(NCORES, alpha, *flags)
    return _CACHE[key]


def _bf(x):
    return np.ascontiguousarray(x.astype(ml_dtypes.bfloat16))


def make_consts(W1, b1, card_g, card_b, carry_g, carry_b, ln_g, ln_b, W2):
    # fold card_g into the cards half of W1; card_b into b1
    W1xh = W1[:D, :]                     # [D, 2D]
    W1ch = card_g[:, None] * W1[D:, :]   # [D, 2D]
    b1f = (b1 + card_b @ W1[D:, :]).astype(np.float32)
    # W12 = W1 @ W2 (+ b1 folded at runtime via b2 path); bump handled apart
    W12x = (W1xh.astype(np.float64) @ W2.astype(np.float64)).astype(
        np.float32)
    W12c = (W1ch.astype(np.float64) @ W2.astype(np.float64)).astype(
        np.float32)
    ut = np.triu(np.ones((P, P), np.float32))
    stm = np.zeros((P, P), np.float32)
    for i in range(1, P):
        stm[i - 1, i] = 1.0
    l0 = np.triu(np.ones((NCH, NCH), np.float32), k=1)
    csel = np.zeros((P, NCH, NCH), np.float32)
    for j in range(NCH):
        csel[:, j, j] = 1.0
    csel = csel.reshape(P, NCH * NCH)
    return {
        "w1x": _bf(np.concatenate([W1xh, W1xh], 0)),
        "w1c": _bf(np.concatenate([W1ch, W1ch], 0)),
        "b1c": b1f[:, None],
        "w12x": _bf(np.concatenate([W12x, W12x], 0)),
        "w12c": _bf(np.concatenate([W12c, W12c], 0)),
        "ut": _bf(ut), "st": _bf(stm), "l0": l0, "csel": _bf(csel),
        "eyer": _bf(np.eye(P, dtype=np.float32)),
        "onesr": _bf(np.ones((1, P), np.float32)),
        "cgr": np.tile(carry_g[None, :], (NCH, 1)).astype(np.float32),
        "cbr": np.tile(carry_b[None, :], (NCH, 1)).astype(np.float32),
        "lgr": np.tile(ln_g[None, :], (P, 1)).astype(np.float32),
        "lbr": np.tile(ln_b[None, :], (P, 1)).astype(np.float32),
    }


def build_all(inputs):
    """Returns (nc, in_maps) for the 8 cores."""
    x = np.ascontiguousarray(np.asarray(inputs["x"], np.float32))
    mark_W = np.asarray(inputs["mark_W"], np.float32)
    mark_b = np.asarray(inputs["mark_b"], np.float32)
    gate_W = np.asarray(inputs["gate_W"], np.float32)
    gate_b = np.asarray(inputs["gate_b"], np.float32)
    carry_g = np.asarray(inputs["carry_g"], np.float32)
    carry_b = np.asarray(inputs["carry_b"], np.float32)
    card_g = np.asarray(inputs["card_g"], np.float32)
    card_b = np.asarray(inputs["card_b"], np.float32)
    W1 = np.asarray(inputs["W1"], np.float32)
    b1 = np.asarray(inputs["b1"], np.float32)
    alpha = float(np.asarray(inputs["alpha"]))
    W2 = np.asarray(inputs["W2"], np.float32)
    b2 = np.asarray(inputs["b2"], np.float32)
    proj_W = np.asarray(inputs["proj_W"], np.float32)
    proj_b = np.asarray(inputs["proj_b"], np.float32)
    ln_g = np.asarray(inputs["ln_g"], np.float32)
    ln_b = np.asarray(inputs["ln_b"], np.float32)

    has_carry_gb = bool(np.any(carry_g != 1.0) or np.any(carry_b != 0.0))
    b1f = b1 + card_b @ W1[D:, :]
    flags = (bool(np.any(mark_b)), bool(np.any(gate_b)), bool(np.any(proj_b)),
             has_carry_gb, bool(np.any(b1f)), bool(np.any(b2)),
             bool(np.any(ln_g != 1.0)), bool(np.any(ln_b)))
    nc = _get_program(alpha, flags)

    common = make_consts(W1, b1, card_g, card_b, carry_g, carry_b,
                         ln_g, ln_b, W2)
    common.update({
        "mkw": _bf(mark_W), "gtw": _bf(gate_W), "pjw": _bf(proj_W),
        "mkb": _bf(mark_b[None, :]), "gtb": _bf(gate_b[None, :]),
        "pjb": _bf(proj_b[None, :]),
        "w2": _bf(W2), "b2c": b2[:, None].astype(np.float32),
    })
    in_maps = []
    for c in range(NCORES):
        b, half = c // 2, c % 2
        xs = x[b, half * R:(half + 1) * R, :]
        m = dict(common)
        m["xn"] = np.ascontiguousarray(xs)
        m["xt"] = _bf(xs.T)
        m["segm"] = np.array([[1.0 - half]], np.float32)
        m["usem"] = np.array([[float(half)]], np.float32)
        in_maps.append(m)
    return nc, in_maps


def kernel(**inputs):
    nc, in_maps = build_all(inputs)
    res = run_bass_kernel_spmd(nc, in_maps, list(range(NCORES)))
    out = np.empty((B, T, C), np.float32)
    for c in range(NCORES):
        b, half = c // 2, c % 2
        out[b, half * R:(half + 1) * R, :] = res.results[c]["y"]
    return out


# revision 18
# speedup vs baseline: 1.0804x; 1.0362x over previous
"""Trainium2 Bass kernel for nn_ChunkedMultiHeadCardPassingLayer.

Sharding: 8 cores = (batch b = core//2) x (T-half = core%2). Each core
processes 2048 contiguous tokens of one batch end-to-end; the only
cross-core dependency is the chunk-carry prefix, resolved with a 4KB
paired AllReduce.

v2: bf16 matmul operands and intermediates (halves weight/x DMA),
W1@W2 fold so the MLP tail is square+exp+one fused DVE op (the +u
pass folds into a precomputed comb@(W1@W2) matmul), residual add on
GPSIMD, xt kept resident across phases (no reload in phase 3), pjw
prefetched before the collective, PSUM evacuated promptly in phase 4.

Self-contained: hardcodes shapes; host-side prep is limited to slicing,
transposes, casts and tiny constant matrices.
"""
import os
os.environ.setdefault("JAX_PLATFORMS", "cpu")

import numpy as np
import ml_dtypes
from contextlib import ExitStack

import concourse.bacc as bacc
import concourse.mybir as mybir
import concourse.tile as tile
from concourse.bass_utils import run_bass_kernel_spmd

F32 = mybir.dt.float32
BF16 = mybir.dt.bfloat16
AX = mybir.AxisListType
ALU = mybir.AluOpType
ACTF = mybir.ActivationFunctionType

# problem constants
B, T, C = 4, 4096, 1024
H, CS = 16, 128
D = C // H            # 64
NCORES = 8
R = T // 2            # 2048 rows per core
NCH = R // CS         # 16 chunks per core
NG = C // 128         # 8 groups of (2 heads x 64)
NPG = NCH // 4        # 4 position groups of 512
EPS = 1e-5
P = 128


def _build(ncores, alpha, has_mark_b, has_gate_b, has_proj_b,
           has_carry_gb, has_b1, has_b2, has_ln_g, has_ln_b):
    assert not has_b1 and not has_b2, "MLP biases folded out; must be zero"
    nc = bacc.Bacc("TRN2", target_bir_lowering=False, debug=False,
                   num_devices=ncores)

    # ---------------- DRAM I/O ----------------
    xt_d = nc.dram_tensor("xt", [C, R], BF16, kind="ExternalInput")
    xn_d = nc.dram_tensor("xn", [R, C], F32, kind="ExternalInput")
    mkw_d = nc.dram_tensor("mkw", [C, C], BF16, kind="ExternalInput")
    gtw_d = nc.dram_tensor("gtw", [C, C], BF16, kind="ExternalInput")
    pjw_d = nc.dram_tensor("pjw", [C, C], BF16, kind="ExternalInput")
    mkb_d = nc.dram_tensor("mkb", [1, C], BF16, kind="ExternalInput")
    gtb_d = nc.dram_tensor("gtb", [1, C], BF16, kind="ExternalInput")
    pjb_d = nc.dram_tensor("pjb", [1, C], BF16, kind="ExternalInput")
    w1x_d = nc.dram_tensor("w1x", [2 * D, 2 * D], BF16, kind="ExternalInput")
    w1c_d = nc.dram_tensor("w1c", [2 * D, 2 * D], BF16, kind="ExternalInput")
    w12x_d = nc.dram_tensor("w12x", [2 * D, D], BF16, kind="ExternalInput")
    w12c_d = nc.dram_tensor("w12c", [2 * D, D], BF16, kind="ExternalInput")
    b1_d = nc.dram_tensor("b1c", [2 * D, 1], F32, kind="ExternalInput")
    w2_d = nc.dram_tensor("w2", [2 * D, D], BF16, kind="ExternalInput")
    b2_d = nc.dram_tensor("b2c", [D, 1], F32, kind="ExternalInput")
    ut_d = nc.dram_tensor("ut", [P, P], BF16, kind="ExternalInput")
    st_d = nc.dram_tensor("st", [P, P], BF16, kind="ExternalInput")
    l0_d = nc.dram_tensor("l0", [NCH, NCH], F32, kind="ExternalInput")
    eye_d = nc.dram_tensor("eyer", [P, P], BF16, kind="ExternalInput")
    csel_d = nc.dram_tensor("csel", [P, NCH * NCH], BF16,
                            kind="ExternalInput")
    onesr_d = nc.dram_tensor("onesr", [1, P], BF16, kind="ExternalInput")
    segm_d = nc.dram_tensor("segm", [1, 1], F32, kind="ExternalInput")
    usem_d = nc.dram_tensor("usem", [1, 1], F32, kind="ExternalInput")
    cgr_d = nc.dram_tensor("cgr", [NCH, D], F32, kind="ExternalInput")
    cbr_d = nc.dram_tensor("cbr", [NCH, D], F32, kind="ExternalInput")
    lgr_d = nc.dram_tensor("lgr", [P, C], F32, kind="ExternalInput")
    lbr_d = nc.dram_tensor("lbr", [P, C], F32, kind="ExternalInput")

    y_d = nc.dram_tensor("y", [R, C], F32, kind="ExternalOutput")

    lc_d = nc.dram_tensor("lc_spill", [R, C], BF16)   # local_cum spill
    cc_in = nc.dram_tensor("cc_in", [1, C], F32)
    cc_out = nc.dram_tensor("cc_out", [1, C], F32)

    groups = ([[i, i + 1] for i in range(0, ncores, 2)]
              if ncores > 1 else [[0]])

    with tile.TileContext(nc) as tc, ExitStack() as top:
        const_p = top.enter_context(tc.tile_pool(name="const", bufs=1))
        carr_p = top.enter_context(tc.tile_pool(name="carr", bufs=1))
        xt_p = top.enter_context(tc.tile_pool(name="xtp", bufs=1))
        pj_p = top.enter_context(tc.tile_pool(name="pjp", bufs=1))

        # ---------- constants ----------
        ut = const_p.tile([P, P], BF16)
        st = const_p.tile([P, P], BF16)
        l0 = const_p.tile([NCH, NCH], F32)
        eyer = const_p.tile([P, P], BF16)
        csel = const_p.tile([P, NCH * NCH], BF16)
        w1x = const_p.tile([2 * D, 2 * D], BF16)
        w1c = const_p.tile([2 * D, 2 * D], BF16)
        w12x = const_p.tile([2 * D, D], BF16)
        w12c = const_p.tile([2 * D, D], BF16)
        b1c = const_p.tile([2 * D, 1], F32)
        w2 = const_p.tile([2 * D, D], BF16)
        b2c = const_p.tile([D, 1], F32)
        segm = const_p.tile([1, 1], F32)
        usem = const_p.tile([1, 1], F32)
        for t_, d_ in ((ut, ut_d), (st, st_d), (l0, l0_d), (eyer, eye_d),
                       (csel, csel_d), (w1x, w1x_d), (w1c, w1c_d),
                       (w12x, w12x_d), (w12c, w12c_d),
                       (b1c, b1_d), (w2, w2_d), (b2c, b2_d), (segm, segm_d),
                       (usem, usem_d)):
            nc.sync.dma_start(t_[:], d_.ap())
        ones1r = const_p.tile([1, P], BF16)
        nc.sync.dma_start(ones1r[:], onesr_d.ap())
        ones1_16 = const_p.tile([1, NCH], F32)
        nc.vector.memset(ones1_16[:], 1.0)
        ones16_1 = const_p.tile([NCH, 1], F32)
        nc.vector.memset(ones16_1[:], 1.0)
        eps128 = const_p.tile([P, 1], F32)
        nc.vector.memset(eps128[:], EPS)
        if has_mark_b or has_gate_b or has_proj_b:
            mkb = const_p.tile([1, C], BF16)
            gtb = const_p.tile([1, C], BF16)
            pjb = const_p.tile([1, C], BF16)
            nc.sync.dma_start(mkb[:], mkb_d.ap())
            nc.sync.dma_start(gtb[:], gtb_d.ap())
            nc.sync.dma_start(pjb[:], pjb_d.ap())
        if has_carry_gb:
            cgr = const_p.tile([NCH, D], F32)
            cbr = const_p.tile([NCH, D], F32)
            nc.sync.dma_start(cgr[:], cgr_d.ap())
            nc.sync.dma_start(cbr[:], cbr_d.ap())

        cs_sb = carr_p.tile([NCH, C], F32)
        ncarry = carr_p.tile([NCH, C], BF16)

        # xt resident across all phases (phase 1 lhsT + phase 3 rhs)
        xt = []
        for g in range(NG):
            t_ = xt_p.tile([P, R], BF16, tag=f"xt{g}", name=f"xt{g}")
            nc.sync.dma_start(t_[:], xt_d.ap()[g * P:(g + 1) * P, :])
            xt.append(t_)

        # ================ phase 1: pm/gate/scan ================
        with tc.tile_pool(name="wgt", bufs=1) as wgt_p, \
             tc.tile_pool(name="ph1", bufs=3) as ph1_p, \
             tc.tile_pool(name="ps1", bufs=2, space="PSUM") as ps1_p, \
             tc.tile_pool(name="pslc", bufs=2, space="PSUM") as pslc_p, \
             tc.tile_pool(name="pscs", bufs=1, space="PSUM") as pscs_p:
            mkw, gtw = [], []
            for k in range(NG):
                mt = wgt_p.tile([P, C], BF16, tag=f"mk{k}", name=f"mk{k}")
                gt_ = wgt_p.tile([P, C], BF16, tag=f"gk{k}", name=f"gk{k}")
                nc.sync.dma_start(mt[:], mkw_d.ap()[k * P:(k + 1) * P, :])
                nc.sync.dma_start(gt_[:], gtw_d.ap()[k * P:(k + 1) * P, :])
                mkw.append(mt)
                gtw.append(gt_)

            cs_ps = pscs_p.tile([NCH, C], F32, tag="csps")
            for j in range(NCH):
                for n in range(2):
                    sl = slice(n * 512, (n + 1) * 512)
                    pm_ps = ps1_p.tile([P, 512], F32, tag="pm", name="pm_ps")
                    gt_ps = ps1_p.tile([P, 512], F32, tag="gt", name="gt_ps")
                    for k in range(NG):
                        lhs = xt[k][:, j * P:(j + 1) * P]
                        st_ = (k == 0)
                        sp = (k == NG - 1) and not (has_mark_b or has_gate_b)
                        nc.tensor.matmul(pm_ps[:], lhs, mkw[k][:, sl],
                                         start=st_, stop=sp)
                        nc.tensor.matmul(gt_ps[:], lhs, gtw[k][:, sl],
                                         start=st_, stop=sp)
                    if has_mark_b or has_gate_b:
                        nc.tensor.matmul(pm_ps[:], ones1r[:], mkb[:, sl],
                                         start=False, stop=True)
                        nc.tensor.matmul(gt_ps[:], ones1r[:], gtb[:, sl],
                                         start=False, stop=True)
                    gates = ph1_p.tile([P, 512], F32, tag="gates",
                                       name="gates")
                    nc.scalar.activation(gates[:], gt_ps[:], ACTF.Sigmoid)
                    gated = ph1_p.tile([P, 512], BF16, tag="gated",
                                       name="gated")
                    nc.vector.tensor_tensor(gated[:], gates[:], pm_ps[:],
                                            op=ALU.mult)
                    nc.tensor.matmul(cs_ps[:, sl],
                                     csel[:, j * NCH:(j + 1) * NCH],
                                     gated[:], start=(j == 0),
                                     stop=(j == NCH - 1))
                    lc_ps = pslc_p.tile([P, 512], F32, tag="lcps",
                                        name="lc_ps")
                    nc.tensor.matmul(lc_ps[:], ut[:], gated[:],
                                     start=True, stop=True)
                    lcs = ph1_p.tile([P, 512], BF16, tag="lcs", name="lcs")
                    nc.scalar.copy(lcs[:], lc_ps[:])
                    nc.sync.dma_start(
                        lc_d.ap()[j * P:(j + 1) * P, sl], lcs[:])
            nc.vector.tensor_copy(cs_sb[:], cs_ps[:])

        # prefetch proj weights + ln gains while collective runs
        pjw = []
        for k in range(NG):
            pt = pj_p.tile([P, C], BF16, tag=f"pj{k}", name=f"pj{k}")
            nc.sync.dma_start(pt[:], pjw_d.ap()[k * P:(k + 1) * P, :])
            pjw.append(pt)
        if has_ln_g:
            lgr = pj_p.tile([P, C], F32)
            nc.sync.dma_start(lgr[:], lgr_d.ap())
        if has_ln_b:
            lbr = pj_p.tile([P, C], F32)
            nc.sync.dma_start(lbr[:], lbr_d.ap())

        # ================ carries + collective ================
        with tc.tile_pool(name="car", bufs=1) as car_p, \
             tc.tile_pool(name="pscar", bufs=1, space="PSUM") as pscar_p:
            tot_ps = pscar_p.tile([1, C], F32, tag="tot")
            for n in range(2):
                sl = slice(n * 512, (n + 1) * 512)
                nc.tensor.matmul(tot_ps[:, sl], ones16_1[:], cs_sb[:, sl],
                                 start=True, stop=True)
            ccin_sb = car_p.tile([1, C], F32)
            nc.vector.tensor_scalar(ccin_sb[:], tot_ps[:], segm[:], None,
                                    op0=ALU.mult)
            nc.sync.dma_start(cc_in.ap(), ccin_sb[:])
            nc.gpsimd.collective_compute(
                "AllReduce", ALU.add, replica_groups=groups,
                ins=[cc_in.ap()], outs=[cc_out.ap()])
            base_sb = car_p.tile([1, C], F32)
            nc.sync.dma_start(base_sb[:], cc_out.ap())
            basem = car_p.tile([1, C], F32)
            nc.vector.tensor_scalar(basem[:], base_sb[:], usem[:], None,
                                    op0=ALU.mult)

            carx_ps = pscar_p.tile([NCH, C], F32, tag="carx")
            for n in range(2):
                sl = slice(n * 512, (n + 1) * 512)
                nc.tensor.matmul(carx_ps[:, sl], l0[:], cs_sb[:, sl],
                                 start=True, stop=False)
                nc.tensor.matmul(carx_ps[:, sl], ones1_16[:], basem[:, sl],
                                 start=False, stop=True)

            # ncarry = LN(carries) over d segments
            carr = car_p.tile([NCH, C], F32)
            nc.vector.tensor_copy(carr[:], carx_ps[:])
            c3 = carr[:].rearrange("p (h d) -> p h d", d=D)
            r1 = car_p.tile([NCH, H], F32)
            nc.vector.tensor_reduce(r1[:], c3, axis=AX.X, op=ALU.add)
            sqc = car_p.tile([NCH, C], F32)
            nc.vector.tensor_tensor(sqc[:], carr[:], carr[:], op=ALU.mult)
            r2 = car_p.tile([NCH, H], F32)
            nc.vector.tensor_reduce(r2[:], sqc[:].rearrange(
                "p (h d) -> p h d", d=D), axis=AX.X, op=ALU.add)
            mu = car_p.tile([NCH, H], F32)
            nc.vector.tensor_scalar(mu[:], r1[:], 1.0 / D, None, op0=ALU.mult)
            em2 = car_p.tile([NCH, H], F32)
            nc.vector.tensor_scalar(em2[:], r2[:], 1.0 / D, None,
                                    op0=ALU.mult)
            musq = car_p.tile([NCH, H], F32)
            nc.vector.tensor_tensor(musq[:], mu[:], mu[:], op=ALU.mult)
            var = car_p.tile([NCH, H], F32)
            nc.vector.tensor_tensor(var[:], em2[:], musq[:], op=ALU.subtract)
            eps16 = car_p.tile([NCH, 1], F32)
            nc.vector.memset(eps16[:], EPS)
            sd = car_p.tile([NCH, H], F32)
            nc.scalar.activation(sd[:], var[:], ACTF.Sqrt, bias=eps16[:])
            rstd = car_p.tile([NCH, H], F32)
            nc.vector.reciprocal(rstd[:], sd[:])
            mu_b = mu[:].unsqueeze(2).to_broadcast([NCH, H, D])
            rstd_b = rstd[:].unsqueeze(2).to_broadcast([NCH, H, D])
            cen = car_p.tile([NCH, C], F32)
            nc.vector.tensor_tensor(cen[:].rearrange("p (h d) -> p h d", d=D),
                                    c3, mu_b, op=ALU.subtract)
            if has_carry_gb:
                nrm = car_p.tile([NCH, C], F32)
                nc.vector.tensor_tensor(
                    nrm[:].rearrange("p (h d) -> p h d", d=D),
                    cen[:].rearrange("p (h d) -> p h d", d=D), rstd_b,
                    op=ALU.mult)
                cg_b = cgr[:].unsqueeze(1).to_broadcast([NCH, H, D])
                cb_b = cbr[:].unsqueeze(1).to_broadcast([NCH, H, D])
                nrm2 = car_p.tile([NCH, C], F32)
                nc.vector.tensor_tensor(
                    nrm2[:].rearrange("p (h d) -> p h d", d=D),
                    nrm[:].rearrange("p (h d) -> p h d", d=D), cg_b,
                    op=ALU.mult)
                nc.vector.tensor_tensor(
                    ncarry[:].rearrange("p (h d) -> p h d", d=D),
                    nrm2[:].rearrange("p (h d) -> p h d", d=D), cb_b,
                    op=ALU.add)
            else:
                nc.vector.tensor_tensor(
                    ncarry[:].rearrange("p (h d) -> p h d", d=D),
                    cen[:].rearrange("p (h d) -> p h d", d=D), rstd_b,
                    op=ALU.mult)

        # ===== phases 2-4, interleaved per position-group of 4 chunks =====
        HH = H // 2  # heads per column half
        with ExitStack() as late:
            big_p = late.enter_context(tc.tile_pool(name="bigpool", bufs=28))
            lcin_p = late.enter_context(tc.tile_pool(name="lcin", bufs=3))
            ph2_p = late.enter_context(tc.tile_pool(name="ph2", bufs=2))
            ph3_p = late.enter_context(tc.tile_pool(name="ph3", bufs=2))
            ph4_p = late.enter_context(tc.tile_pool(name="ph4", bufs=2))
            ps2_p = late.enter_context(
                tc.tile_pool(name="ps2", bufs=2, space="PSUM"))
            pstr_p = late.enter_context(
                tc.tile_pool(name="pstr", bufs=1, space="PSUM"))
            ps3_p = late.enter_context(
                tc.tile_pool(name="ps3", bufs=2, space="PSUM"))
            ps3b_p = late.enter_context(
                tc.tile_pool(name="ps3b", bufs=1, space="PSUM"))
            ps4_p = late.enter_context(
                tc.tile_pool(name="ps4", bufs=1, space="PSUM"))

            for pg in range(NPG):
                psl = slice(pg * 512, (pg + 1) * 512)
                cardsT = [None] * NG
                outT = [None] * NG
                for g in range(NG):
                    cardsT[g] = big_p.tile([P, 512], BF16, tag="bigtile",
                                           name=f"cardsT{pg}_{g}")

                # ---- phase 2: cards for the 4 chunks of this pg ----
                for jj in range(4):
                    j = pg * 4 + jj
                    ncrow = lcin_p.tile([1, C], BF16, tag="ncrow",
                                        name="ncrow", bufs=2)
                    nc.sync.dma_start(ncrow[:], ncarry[j:j + 1, :])
                    for n in range(2):
                        sl = slice(n * 512, (n + 1) * 512)
                        lcj = lcin_p.tile([P, 512], BF16, tag="lcin",
                                          name="lcj", bufs=4)
                        nc.sync.dma_start(lcj[:],
                                          lc_d.ap()[j * P:(j + 1) * P, sl])
                        cl_ps = ps2_p.tile([P, 512], F32, tag="clps",
                                           name="cl_ps")
                        nc.tensor.matmul(cl_ps[:], st[:], lcj[:],
                                         start=True, stop=False)
                        nc.tensor.matmul(cl_ps[:], ones1r[:],
                                         ncrow[0:1, sl],
                                         start=False, stop=True)
                        # segmented LN over d
                        sq = ph2_p.tile([P, 512], BF16, tag="sq", name="sq")
                        nc.scalar.square(sq[:], cl_ps[:])
                        r1c = ph2_p.tile([P, HH], F32, tag="r1c", name="r1c")
                        nc.vector.tensor_reduce(
                            r1c[:],
                            cl_ps[:].rearrange("p (h d) -> p h d", d=D),
                            axis=AX.X, op=ALU.add)
                        r2c = ph2_p.tile([P, HH], F32, tag="r2c", name="r2c")
                        nc.vector.tensor_reduce(
                            r2c[:], sq[:].rearrange("p (h d) -> p h d", d=D),
                            axis=AX.X, op=ALU.add)
                        muc = ph2_p.tile([P, HH], F32, tag="muc",
                                         name="muc")
                        nc.vector.tensor_scalar(muc[:], r1c[:], 1.0 / D,
                                                None, op0=ALU.mult)
                        musqc = ph2_p.tile([P, HH], F32, tag="musqc",
                                           name="musqc")
                        nc.vector.tensor_tensor(musqc[:], muc[:], muc[:],
                                                op=ALU.mult)
                        varc = ph2_p.tile([P, HH], F32, tag="varc",
                                          name="varc")
                        nc.vector.scalar_tensor_tensor(
                            varc[:], r2c[:], 1.0 / D, musqc[:],
                            op0=ALU.mult, op1=ALU.subtract)
                        sdc = ph2_p.tile([P, HH], F32, tag="sdc",
                                         name="sdc")
                        nc.scalar.activation(sdc[:], varc[:], ACTF.Sqrt,
                                             bias=eps128[:])
                        rstdc = ph2_p.tile([P, HH], F32, tag="rstdc",
                                           name="rstdc")
                        nc.vector.reciprocal(rstdc[:], sdc[:])
                        mu_bc = muc[:].unsqueeze(2).to_broadcast([P, HH, D])
                        rstd_bc = rstdc[:].unsqueeze(2).to_broadcast(
                            [P, HH, D])
                        cenc = ph2_p.tile([P, 512], BF16, tag="cenc",
                                          name="cenc")
                        nc.vector.tensor_tensor(
                            cenc[:].rearrange("p (h d) -> p h d", d=D),
                            cl_ps[:].rearrange("p (h d) -> p h d", d=D),
                            mu_bc, op=ALU.subtract)
                        cards = ph2_p.tile([P, 512], BF16, tag="cards",
                                           name="cards")
                        nc.vector.tensor_tensor(
                            cards[:].rearrange("p (h d) -> p h d", d=D),
                            cenc[:].rearrange("p (h d) -> p h d", d=D),
                            rstd_bc, op=ALU.mult)
                        for gg in range(4):
                            g = n * 4 + gg
                            tr_ps = pstr_p.tile([P, P], BF16, tag="trps",
                                                name="tr_ps")
                            nc.tensor.transpose(
                                tr_ps[:], cards[:, gg * P:(gg + 1) * P],
                                eyer[:])
                            if gg % 2 == 0:
                                nc.scalar.copy(
                                    cardsT[g][:, jj * P:(jj + 1) * P],
                                    tr_ps[:])
                            else:
                                nc.vector.tensor_copy(
                                    cardsT[g][:, jj * P:(jj + 1) * P],
                                    tr_ps[:])

                # ---- phase 3: head MLP for this pg ----
                # ho = comb @ (W1@W2) + bump @ W2,
                # bump = alpha * u * exp(-u^2/2), u = comb @ W1 (+b1)
                for g in range(NG):
                    outT[g] = big_p.tile([P, 512], BF16, tag="bigtile",
                                         name=f"outT{pg}_{g}")
                for g in range(NG):
                    o2_ps = ps3b_p.tile([P, 512], F32, tag="o2",
                                        name="o2_ps")
                    for hh in range(2):
                        h = 2 * g + hh
                        off = hh * D
                        xg_r = xt[g][off:off + D, psl]
                        cd_r = cardsT[g][off:off + D, :]
                        h1_ps = ps3_p.tile([P, 512], F32, tag="h1",
                                           name="h1_ps")
                        nc.tensor.matmul(h1_ps[:], w1x[off:off + D, :], xg_r,
                                         start=True, stop=False)
                        nc.tensor.matmul(h1_ps[:], w1c[off:off + D, :], cd_r,
                                         start=False, stop=True)
                        sq3 = ph3_p.tile([P, 512], BF16, tag="sq3",
                                         name="sq3")
                        nc.scalar.square(sq3[:], h1_ps[:])
                        e3 = ph3_p.tile([P, 512], BF16, tag="e3", name="e3")
                        nc.scalar.activation(e3[:], sq3[:], ACTF.Exp,
                                             scale=-0.5)
                        wb = ph3_p.tile([P, 512], BF16, tag="wb", name="wb")
                        nc.vector.scalar_tensor_tensor(
                            wb[:], e3[:], float(alpha), h1_ps[:],
                            op0=ALU.mult, op1=ALU.mult)
                        o2v = o2_ps[off:off + D, :]
                        nc.tensor.matmul(o2v, w12x[off:off + D, :], xg_r,
                                         start=True, stop=False)
                        nc.tensor.matmul(o2v, w12c[off:off + D, :], cd_r,
                                         start=False, stop=False)
                        nc.tensor.matmul(o2v, w2[:], wb[:],
                                         start=False, stop=True)
                    nc.vector.tensor_copy(outT[g][:], o2_ps[:])

                # ---- phase 4: proj + LN + residual for this pg ----
                for tt in range(4):
                    t_i = pg * 4 + tt
                    col = tt * P
                    y_ps = ps4_p.tile([P, C], F32, tag="yps", name="y_ps")
                    for k in range(NG):
                        lhs = outT[k][:, col:col + P]
                        st_ = (k == 0)
                        sp = (k == NG - 1) and not has_proj_b
                        for n in range(2):
                            sl = slice(n * 512, (n + 1) * 512)
                            nc.tensor.matmul(y_ps[:, sl], lhs, pjw[k][:, sl],
                                             start=st_, stop=sp)
                    if has_proj_b:
                        for n in range(2):
                            sl = slice(n * 512, (n + 1) * 512)
                            nc.tensor.matmul(y_ps[:, sl], ones1r[:],
                                             pjb[:, sl],
                                             start=False, stop=True)
                    y_raw = ph4_p.tile([P, C], F32, tag="yraw", name="y_raw")
                    nc.vector.tensor_copy(y_raw[:], y_ps[:])
                    s1 = ph4_p.tile([P, 1], F32, tag="s1", name="s1")
                    jk1 = ph4_p.tile([P, C], BF16, tag="jk1", name="jk1",
                                     bufs=1)
                    nc.scalar.activation(jk1[:], y_raw[:], ACTF.Copy,
                                         accum_out=s1[:])
                    sc4 = ph4_p.tile([P, C], BF16, tag="sc4", name="sc4",
                                     bufs=1)
                    s2 = ph4_p.tile([P, 1], F32, tag="s2", name="s2")
                    nc.scalar.activation(sc4[:], y_raw[:], ACTF.Square,
                                         scale=1.0 / 32.0, accum_out=s2[:])
                    m1 = ph4_p.tile([P, 1], F32, tag="m1", name="m1")
                    nc.vector.tensor_scalar(m1[:], s1[:], 1.0 / C, None,
                                            op0=ALU.mult)
                    msq = ph4_p.tile([P, 1], F32, tag="msq", name="msq")
                    nc.vector.tensor_tensor(msq[:], m1[:], m1[:],
                                            op=ALU.mult)
                    var4 = ph4_p.tile([P, 1], F32, tag="var4", name="var4")
                    nc.vector.tensor_tensor(var4[:], s2[:], msq[:],
                                            op=ALU.subtract)
                    sd4 = ph4_p.tile([P, 1], F32, tag="sd4", name="sd4")
                    nc.scalar.activation(sd4[:], var4[:], ACTF.Sqrt,
                                         bias=eps128[:])
                    rstd4 = ph4_p.tile([P, 1], F32, tag="rstd4",
                                       name="rstd4")
                    nc.vector.reciprocal(rstd4[:], sd4[:])
                    tnorm = ph4_p.tile([P, C], F32, tag="tnorm",
                                       name="tnorm")
                    nc.vector.tensor_scalar(tnorm[:], y_raw[:], m1[:],
                                            rstd4[:], op0=ALU.subtract,
                                            op1=ALU.mult)
                    if has_ln_g:
                        nc.vector.tensor_tensor(tnorm[:], tnorm[:], lgr[:],
                                                op=ALU.mult)
                    if has_ln_b:
                        nc.vector.tensor_tensor(tnorm[:], tnorm[:], lbr[:],
                                                op=ALU.add)
                    xa = ph4_p.tile([P, C], F32, tag="xa", name="xa")
                    nc.sync.dma_start(xa[:],
                                      xn_d.ap()[t_i * P:(t_i + 1) * P, :])
                    yout = ph4_p.tile([P, C], F32, tag="yout", name="yout")
                    nc.gpsimd.tensor_tensor(yout[:], tnorm[:], xa[:],
                                            op=ALU.add)
                    nc.sync.dma_start(y_d.ap()[t_i * P:(t_i + 1) * P, :],
                                      yout[:])

    nc.compile()
    return nc


_CACHE = {}


def _get_program(alpha, flags):
    key = (alpha, flags)
    if key not in _CACHE:
        _CACHE[key] = _build(NCORES, alpha, *flags)
    return _CACHE[key]


def _bf(x):
    return np.ascontiguousarray(x.astype(ml_dtypes.bfloat16))


def make_consts(W1, b1, card_g, card_b, carry_g, carry_b, ln_g, ln_b, W2):
    # fold card_g into the cards half of W1; card_b into b1
    W1xh = W1[:D, :]                     # [D, 2D]
    W1ch = card_g[:, None] * W1[D:, :]   # [D, 2D]
    b1f = (b1 + card_b @ W1[D:, :]).astype(np.float32)
    # W12 = W1 @ W2 (+ b1 folded at runtime via b2 path); bump handled apart
    W12x = (W1xh.astype(np.float64) @ W2.astype(np.float64)).astype(
        np.float32)
    W12c = (W1ch.astype(np.float64) @ W2.astype(np.float64)).astype(
        np.float32)
    ut = np.triu(np.ones((P, P), np.float32))
    stm = np.zeros((P, P), np.float32)
    for i in range(1, P):
        stm[i - 1, i] = 1.0
    l0 = np.triu(np.ones((NCH, NCH), np.float32), k=1)
    csel = np.zeros((P, NCH, NCH), np.float32)
    for j in range(NCH):
        csel[:, j, j] = 1.0
    csel = csel.reshape(P, NCH * NCH)
    return {
        "w1x": _bf(np.concatenate([W1xh, W1xh], 0)),
        "w1c": _bf(np.concatenate([W1ch, W1ch], 0)),
        "b1c": b1f[:, None],
        "w12x": _bf(np.concatenate([W12x, W12x], 0)),
        "w12c": _bf(np.concatenate([W12c, W12c], 0)),
        "ut": _bf(ut), "st": _bf(stm), "l0": l0, "csel": _bf(csel),
        "eyer": _bf(np.eye(P, dtype=np.float32)),
        "onesr": _bf(np.ones((1, P), np.float32)),
        "cgr": np.tile(carry_g[None, :], (NCH, 1)).astype(np.float32),
        "cbr": np.tile(carry_b[None, :], (NCH, 1)).astype(np.float32),
        "lgr": np.tile(ln_g[None, :], (P, 1)).astype(np.float32),
        "lbr": np.tile(ln_b[None, :], (P, 1)).astype(np.float32),
    }


def build_all(inputs):
    """Returns (nc, in_maps) for the 8 cores."""
    x = np.ascontiguousarray(np.asarray(inputs["x"], np.float32))
    mark_W = np.asarray(inputs["mark_W"], np.float32)
    mark_b = np.asarray(inputs["mark_b"], np.float32)
    gate_W = np.asarray(inputs["gate_W"], np.float32)
    gate_b = np.asarray(inputs["gate_b"], np.float32)
    carry_g = np.asarray(inputs["carry_g"], np.float32)
    carry_b = np.asarray(inputs["carry_b"], np.float32)
    card_g = np.asarray(inputs["card_g"], np.float32)
    card_b = np.asarray(inputs["card_b"], np.float32)
    W1 = np.asarray(inputs["W1"], np.float32)
    b1 = np.asarray(inputs["b1"], np.float32)
    alpha = float(np.asarray(inputs["alpha"]))
    W2 = np.asarray(inputs["W2"], np.float32)
    b2 = np.asarray(inputs["b2"], np.float32)
    proj_W = np.asarray(inputs["proj_W"], np.float32)
    proj_b = np.asarray(inputs["proj_b"], np.float32)
    ln_g = np.asarray(inputs["ln_g"], np.float32)
    ln_b = np.asarray(inputs["ln_b"], np.float32)

    has_carry_gb = bool(np.any(carry_g != 1.0) or np.any(carry_b != 0.0))
    b1f = b1 + card_b @ W1[D:, :]
    flags = (bool(np.any(mark_b)), bool(np.any(gate_b)), bool(np.any(proj_b)),
             has_carry_gb, bool(np.any(b1f)), bool(np.any(b2)),
             bool(np.any(ln_g != 1.0)), bool(np.any(ln_b)))
    nc = _get_program(alpha, flags)

    common = make_consts(W1, b1, card_g, card_b, carry_g, carry_b,
                         ln_g, ln_b, W2)
    common.update({
        "mkw": _bf(mark_W), "gtw": _bf(gate_W), "pjw": _bf(proj_W),
        "mkb": _bf(mark_b[None, :]), "gtb": _bf(gate_b[None, :]),
        "pjb": _bf(proj_b[None, :]),
        "w2": _bf(W2), "b2c": b2[:, None].astype(np.float32),
    })
    in_maps = []
    for c in range(NCORES):
        b, half = c // 2, c % 2
        xs = x[b, half * R:(half + 1) * R, :]
        m = dict(common)
        m["xn"] = np.ascontiguousarray(xs)
        m["xt"] = _bf(xs.T)
        m["segm"] = np.array([[1.0 - half]], np.float32)
        m["usem"] = np.array([[float(half)]], np.float32)
        in_maps.append(m)
    return nc, in_maps


def kernel(**inputs):
    nc, in_maps = build_all(inputs)
    res = run_bass_kernel_spmd(nc, in_maps, list(range(NCORES)))
    out = np.empty((B, T, C), np.float32)
    for c in range(NCORES):
        b, half = c // 2, c % 2
        out[b, half * R:(half + 1) * R, :] = res.results[c]["y"]
    return out
